# revision 1
# baseline (speedup 1.0000x reference)
"""Bass/Trainium2 kernel for nn_GPREDecoder (GlobalPointer relation-extraction loss).

Strategy: data-parallel over batch (B=8 -> 8 cores, 1 example per core).
The device computes, per example:
  - projT = W_all @ x_aug.T  (channel-major projection, bias folded in)
  - RoPE rotation for the two "ent" heads (J-matmul + cos/sin elementwise)
  - per-head S x S logits tiles on PE, exp(SCALE*logit) on ACT with fused
    per-row accumulation -> per-head sum(exp(masked logits))  (never
    materializing the S x S tensors in HBM)
  - outputs the per-head exp-sums and the final q/k tensors
The host gathers the 64 ground-truth pairs per head from q/k, applies the
multilabel-CE pos/neg log corrections in float64, and returns the scalar loss.
"""

import ml_dtypes
import numpy as np
from contextlib import ExitStack

import concourse.bass as bass
import concourse.mybir as mybir
import concourse.tile as tile
from concourse import bacc
from concourse.bass_utils import run_bass_kernel_spmd

B, S, HID, LAB = 8, 1024, 1024, 64
HD = 68
SCALE = 1.0 / HD**0.5
INF = 1.0e12
NCORES = 8
KPAD = 1152  # 9 * 128 contraction rows (1088 channels + 1 bias row + pad)
MTOT = 544   # total projection output channels
NEG_BIG = -1.0e9  # additive pre-scale mask; exp(SCALE*NEG_BIG) == 0 in fp32

# group order: q_ent0 k_ent0 q_ent1 k_ent1 q_head k_head q_tail k_tail
_GROUP_ORIG = [0, 68, 136, 204, 272, 340, 408, 476]
# heads: (q_group, k_group, tril?)
_HEADS = [(0, 1, True), (2, 3, True), (4, 5, False), (6, 7, False)]


def _spill_slots():
    """Destination (tile, row) slots for the 4 spill groups, in order."""
    slots = []
    for t in range(4):
        slots.extend((t, r) for r in range(68, 128))
    slots.extend((4, r) for r in range(32))
    return slots


def _build_perm():
    """perm[c_new] = original channel index, for the projection output layout."""
    perm = np.zeros(MTOT, np.int64)
    for g in range(4):  # rope groups aligned at row 0 of tiles 0..3
        perm[g * 128: g * 128 + 68] = np.arange(_GROUP_ORIG[g], _GROUP_ORIG[g] + 68)
    slots = _spill_slots()
    pos = 0
    for g in range(4, 8):
        for j in range(68):
            t, r = slots[pos]
            perm[t * 128 + r] = _GROUP_ORIG[g] + j
            pos += 1
    return perm


def _spill_pieces():
    """Per spill group: contiguous (src_tile, src_row0, cnt, dst_row0) DMA pieces."""
    slots = _spill_slots()
    out = {g: [] for g in range(4, 8)}
    pos = 0
    for g in range(4, 8):
        j = 0
        while j < 68:
            t, r = slots[pos]
            cnt = 1
            while j + cnt < 68 and pos + cnt < len(slots) and \
                    slots[pos + cnt] == (t, r + cnt):
                cnt += 1
            out[g].append((t, r, cnt, j))
            pos += cnt
            j += cnt
    return out


def _round_chunks(mtiles):
    """Chunk m-tiles of one [128,1024] psum round into bank-fitting matmul chunks.

    mtiles: [(m, local_start, width)] with local starts such that every
    <=512 chunk stays inside one 512-col bank. Returns
    [(m, local_off, src_off, n)] and the single contiguous ACT span end.
    """
    chunks = []
    for (m, lo, w) in mtiles:
        off = 0
        while off < w:
            n = min(512 - ((lo + off) % 512), w - off)
            chunks.append((m, lo + off, off, n))
            off += n
    return chunks


def _head_rounds(is_tril):
    """Per head: list of rounds; each round = (mtiles, span_end).

    Rounds target [128, 1024] (2-bank) psum tiles. For tril heads the
    m-tile widths shrink (only columns >= 128*m are live), so later
    m-tiles are packed two per round; spans stay contiguous from 0.
    """
    if not is_tril:
        return [([(m, 0, 1024)], 1024) for m in range(8)]
    widths = [1024 - 128 * m for m in range(8)]
    rounds = []
    for group in ((0,), (1,), (2, 6), (3, 7), (4, 5)):
        mtiles = []
        local = 0
        for m in group:
            mtiles.append((m, local, widths[m]))
            local += widths[m]
        rounds.append((mtiles, local))
    return rounds


def _n_act_cols(is_tril):
    return len(_head_rounds(is_tril))


_ACC_COLS = [_n_act_cols(t) for _, _, t in _HEADS]          # per head
_ACC_OFF = np.concatenate([[0], np.cumsum(_ACC_COLS)])      # col offset per head
SUMS_COLS = int(_ACC_OFF[-1])                               # total accum columns


def _build_nc():
    f32 = mybir.dt.float32
    # float32r: same 4-byte storage, but the PE streams it at full rate
    # (strict fp32 runs as 2 half-speed passes = 4x slower). The q/k logits
    # path jmat -> dense -> qk is typed f32r end-to-end. The projection
    # inputs are bf16 to halve the HBM load volume.
    f32r = mybir.dt.float32r
    bf16 = mybir.dt.bfloat16
    Exp = mybir.ActivationFunctionType.Exp

    nc = bacc.Bacc("TRN2", target_bir_lowering=False)

    xT = nc.dram_tensor("xT", [KPAD, S], bf16, kind="ExternalInput")
    wtb = nc.dram_tensor("wtb", [KPAD, MTOT], bf16, kind="ExternalInput")
    trig = nc.dram_tensor("trig", [HD, 2 * S], f32, kind="ExternalInput")
    jtril = nc.dram_tensor("jtril", [128, 256], f32r, kind="ExternalInput")
    sums = nc.dram_tensor("sums", [128, SUMS_COLS], f32, kind="ExternalOutput")
    qkout = nc.dram_tensor("qkout", [8, HD, S], f32r, kind="ExternalOutput")

    xT_r = xT.rearrange("(o p) f -> p o f", p=128)    # [128, 9, 1024]
    wtb_r = wtb.rearrange("(o p) f -> p o f", p=128)  # [128, 9, 544]
    KT_CHUNKS = [(0, 2), (2, 3), (5, 4)]              # (kt0, n_kt) DMA chunks

    with tile.TileContext(nc) as tc, ExitStack() as ctx:
        singles = ctx.enter_context(tc.tile_pool(name="singles", bufs=1))
        scratch = ctx.enter_context(tc.tile_pool(name="scratch", bufs=2))

        xT_sb = singles.tile([128, 9, S], bf16, tag="xT_sb", name="xT_sb")
        wtb_sb = singles.tile([128, 9, MTOT], bf16, tag="wtb_sb", name="wtb_sb")
        trig_sb = singles.tile([HD, 2 * S], f32, tag="trig_sb", name="trig_sb")
        jtril_sb = singles.tile([128, 256], f32r, tag="jtril_sb", name="jtril_sb")
        dense = [singles.tile([128, S], f32r, tag=f"dense{t}", name=f"dense{t}")
                 for t in range(5)]
        qk = [singles.tile([HD, S], f32r, tag=f"qk{g}", name=f"qk{g}")
              for g in range(8)]
        sums_sb = singles.tile([128, SUMS_COLS], f32, tag="sums_sb", name="sums_sb")
        dummy = singles.tile([1, 8], f32, tag="dummy", name="dummy")

        cos_sb = trig_sb[:, 0:S]
        sin_sb = trig_sb[:, S:2 * S]
        jmat_sb = jtril_sb[:, 0:128]
        tril_sb = jtril_sb[:, 128:256].bitcast(f32)

        # Early: zero accumulators; pre-warm the ACT exp table load.
        nc.vector.memset(sums_sb[:], 0.0)
        nc.vector.memset(dummy[:], 0.0)
        nc.scalar.activation(dummy[:], dummy[:], Exp)

        # input DMAs: first kt chunk first so the projection starts ASAP;
        # constants (needed only ~10us in) after the first chunk.
        def in_chunk(ci):
            kt0, nkt = KT_CHUNKS[ci]
            nc.sync.dma_start(out=wtb_sb[:, kt0:kt0 + nkt],
                              in_=wtb_r[:, kt0:kt0 + nkt])
            nc.scalar.dma_start(out=xT_sb[:, kt0:kt0 + nkt],
                                in_=xT_r[:, kt0:kt0 + nkt])

        in_chunk(0)
        nc.sync.dma_start(out=jtril_sb[:], in_=jtril[:, :])
        nc.scalar.dma_start(out=trig_sb[:], in_=trig[:, :])
        in_chunk(1)
        in_chunk(2)

        ps = ctx.enter_context(tc.tile_pool(name="ps", bufs=4, space="PSUM"))

        def proj_tile(t, pt, kt_lo=0, kt_hi=9):
            lo = t * 128
            hi = min(lo + 128, MTOT)
            for kt in range(kt_lo, kt_hi):
                for c in (0, 512):
                    nc.tensor.matmul(
                        pt[0:hi - lo, c:c + 512],
                        wtb_sb[:, kt, lo:hi],
                        xT_sb[:, kt, c:c + 512],
                        start=(kt == 0), stop=(kt == 8),
                    )

        def evac(t, pt, eng):
            hi = min(128, MTOT - t * 128)
            if eng == "act":
                nc.scalar.copy(out=dense[t][0:hi, :], in_=pt[0:hi, :])
            else:
                nc.vector.tensor_copy(out=dense[t][0:hi, :], in_=pt[0:hi, :])

        def jrot(g):
            """J-matmul for rope group g; returns the psum tile to release."""
            pj = ps.tile([128, S], f32, tag="ps", name=f"jq{g}")
            for c in (0, 512):
                nc.tensor.matmul(pj[:, c:c + 512], jmat_sb,
                                 dense[g][:, c:c + 512], start=True, stop=True)
            return pj

        def rope(g, pj):
            # qk[g] = dense[g]*cos + (J @ dense[g])*sin
            nc.gpsimd.tensor_tensor(qk[g][:, :], dense[g][0:HD, :], cos_sb,
                                    mybir.AluOpType.mult)
            rtmp = scratch.tile([HD, S], f32, tag="rtmp", name=f"rtmp{g}")
            nc.vector.tensor_tensor(rtmp[:, :], pj[0:HD, :], sin_sb,
                                    mybir.AluOpType.mult)
            nc.vector.tensor_tensor(qk[g][:, :], qk[g][:, :], rtmp[:, :],
                                    mybir.AluOpType.add)

        def head_logits(h, interleave=None):
            gq, gk, is_tril = _HEADS[h]
            acc = int(_ACC_OFF[h])
            for ri, (mtiles, span_end) in enumerate(_head_rounds(is_tril)):
                pl = ps.tile([128, S], f32, tag="ps", name=f"l{h}_{ri}")
                for (m, lo, so, n) in _round_chunks(mtiles):
                    g0 = 128 * m if is_tril else 0
                    nc.tensor.matmul(
                        pl[:, lo:lo + n],
                        qk[gq][:, m * 128:(m + 1) * 128],
                        qk[gk][:, g0 + so:g0 + so + n],
                        start=True, stop=True,
                    )
                if is_tril:
                    for (m, lo, w) in mtiles:
                        nc.vector.tensor_tensor(
                            pl[:, lo:lo + 128], pl[:, lo:lo + 128],
                            tril_sb, mybir.AluOpType.add)
                nc.scalar.activation(
                    pl[:, 0:span_end], pl[:, 0:span_end], Exp, scale=SCALE,
                    accum_out=sums_sb[:, acc:acc + 1])
                acc += 1
                if interleave is not None:
                    interleave(ri)
            assert acc == int(_ACC_OFF[h + 1])

        # ---- phase B1: projection tiles 0,1 (the ent-h0 rope groups) ----
        pt0 = ps.tile([128, S], f32, tag="ps", name="proj0")
        pt1 = ps.tile([128, S], f32, tag="ps", name="proj1")
        for kt in range(9):
            for t, pt in ((0, pt0), (1, pt1)):
                for c in (0, 512):
                    nc.tensor.matmul(pt[:, c:c + 512],
                                     wtb_sb[:, kt, t * 128:(t + 1) * 128],
                                     xT_sb[:, kt, c:c + 512],
                                     start=(kt == 0), stop=(kt == 8))
        evac(0, pt0, "dve")
        evac(1, pt1, "dve")
        pj0 = jrot(0)
        pj1 = jrot(1)
        rope(0, pj0)
        rope(1, pj1)
        nc.sync.dma_start(out=qkout[0], in_=qk[0][:, :])
        nc.scalar.dma_start(out=qkout[1], in_=qk[1][:, :])

        # ---- ent head 0: starts the ACT exp stream as early as possible ----
        head_logits(0)

        # ---- phase B2: projection tiles 2,3 ----
        pt2 = ps.tile([128, S], f32, tag="ps", name="proj2")
        pt3 = ps.tile([128, S], f32, tag="ps", name="proj3")
        proj_tile(2, pt2)
        proj_tile(3, pt3)
        evac(2, pt2, "dve")
        evac(3, pt3, "dve")

        # ---- phase B3: projection tile 4 + spill regroup for head/tail ----
        pt4 = ps.tile([128, S], f32, tag="ps", name="proj4")
        proj_tile(4, pt4)
        evac(4, pt4, "dve")
        for g, pieces in _spill_pieces().items():
            for i, (t, r0, cnt, d0) in enumerate(pieces):
                eng = nc.sync if (g + i) % 2 == 0 else nc.scalar
                eng.dma_start(out=qk[g][d0:d0 + cnt, :],
                              in_=dense[t][r0:r0 + cnt, :])
            eng = nc.sync if g % 2 == 0 else nc.scalar
            eng.dma_start(out=qkout[g], in_=qk[g][:, :])

        # ---- rope for ent head 1 while the head/tail spill DMAs run ----
        pj2 = jrot(2)
        pj3 = jrot(3)
        rope(2, pj2)
        rope(3, pj3)
        nc.sync.dma_start(out=qkout[2], in_=qk[2][:, :])
        nc.scalar.dma_start(out=qkout[3], in_=qk[3][:, :])

        # ---- remaining heads: head first (its deps finish earliest) ----
        head_logits(2)
        head_logits(1)
        head_logits(3)

        nc.sync.dma_start(out=sums[:, :], in_=sums_sb[:, :])

    nc.finalize()
    return nc


_NC_CACHE = None


def _get_nc():
    global _NC_CACHE
    if _NC_CACHE is None:
        _NC_CACHE = _build_nc()
    return _NC_CACHE


def _host_tables():
    pos = np.arange(S, dtype=np.float64)[:, None]
    inv = np.power(10000.0, -2.0 * np.arange(HD // 2, dtype=np.float64) / HD)
    ang = pos * inv                                   # [S, 34]
    trig = np.zeros((HD, 2 * S), np.float32)
    trig[:, 0:S] = np.repeat(np.cos(ang), 2, axis=1).T
    trig[:, S:2 * S] = np.repeat(np.sin(ang), 2, axis=1).T
    jtril = np.zeros((128, 256), np.float32)          # [:, :128]=J.T, [:, 128:]=tril
    for i in range(HD // 2):
        # J[2i, 2i+1] = -1 ; J[2i+1, 2i] = +1  -> stored transposed
        jtril[2 * i + 1, 2 * i] = -1.0
        jtril[2 * i, 2 * i + 1] = 1.0
    jtril[:, 128:256] = np.where(
        np.arange(128)[None, :] >= np.arange(128)[:, None], 0.0, NEG_BIG)
    return trig, jtril


def _mcce_host(E_dev, q, k, gt):
    """pos/neg multilabel-CE for one (example, head). q,k: [68,S] f32; gt: [P,2]."""
    i = gt[:, 0].astype(np.int64)
    j = gt[:, 1].astype(np.int64)
    flat = i * S + j
    lv = np.sum(q[:, i].astype(np.float64) * k[:, j].astype(np.float64),
                axis=0) * SCALE                       # [P]
    live = flat != 0
    pos_loss = np.log1p(np.sum(np.exp(-lv[live])))
    l00 = float(np.sum(q[:, 0].astype(np.float64) * k[:, 0].astype(np.float64))
                * SCALE)
    uf, ui = np.unique(flat, return_index=True)
    keep = uf != 0
    excl = np.exp(l00) + np.sum(np.exp(lv[ui[keep]]))
    neg_loss = np.log1p(E_dev - excl)
    return pos_loss + neg_loss


def _reference_numpy(hidden, entity_labels, attention_mask, gt_entity, gt_head,
                     gt_tail, ent_emb, W_ent, b_ent, W_head, b_head, W_tail,
                     b_tail):
    """Slow exact numpy fallback (used only if attention_mask is not all-ones)."""
    x = np.concatenate([hidden, ent_emb[entity_labels]], axis=-1)

    def rope(v):
        b, s, h, d = v.shape
        pos = np.arange(s, dtype=np.float32)[:, None]
        inv = np.power(10000.0, -2.0 * np.arange(d // 2, dtype=np.float32) / d)
        ang = pos * inv
        sin = np.repeat(np.sin(ang), 2, axis=-1)[None, :, None, :]
        cos = np.repeat(np.cos(ang), 2, axis=-1)[None, :, None, :]
        v2 = np.stack([-v[..., 1::2], v[..., ::2]], axis=-1).reshape(v.shape)
        return v * cos + v2 * sin

    def gp(x, W, b, mask, heads, use_rope, tril):
        bx, sx, _ = x.shape
        proj = (x @ W.T + b).reshape(bx, sx, heads, 2 * HD)
        qw, kw = proj[..., :HD], proj[..., HD:]
        if use_rope:
            qw, kw = rope(qw), rope(kw)
        logits = np.einsum('bmhd,bnhd->bhmn', qw, kw) * SCALE
        pad = mask[:, None, None, :]
        logits = logits * pad - (1.0 - pad) * INF
        if tril:
            logits = logits - np.tril(np.ones((sx, sx), np.float32), -1) * INF
        return logits

    def mcce(y_true, y_pred):
        bx, hx, sx, _ = y_pred.shape
        flat = y_true[..., 0].astype(np.int64) * sx + y_true[..., 1]
        yp = y_pred.reshape(bx, hx, sx * sx).astype(np.float64)
        total = 0.0
        for b in range(bx):
            for h in range(hx):
                f = flat[b, h]
                live = f != 0
                lv = yp[b, h][f]
                pos = np.log1p(np.sum(np.exp(-lv[live])))
                neg_terms = yp[b, h].copy()
                neg_terms[0] = -np.inf
                neg_terms[np.unique(f)] = -np.inf
                neg = np.log1p(np.sum(np.exp(neg_terms)))
                total += pos + neg
        return total

    loss = 0.0
    loss += mcce(gt_entity, gp(x, W_ent, b_ent, attention_mask, 2, True, True))
    loss += mcce(gt_head, gp(x, W_head, b_head, attention_mask, 1, False, False))
    loss += mcce(gt_tail, gp(x, W_tail, b_tail, attention_mask, 1, False, False))
    return np.array(loss, dtype=np.float32)


def kernel(hidden, entity_labels, attention_mask, gt_entity, gt_head, gt_tail,
           ent_emb, W_ent, b_ent, W_head, b_head, W_tail, b_tail,
           _want_trace=False):
    hidden = np.asarray(hidden, np.float32)
    entity_labels = np.asarray(entity_labels)
    attention_mask = np.asarray(attention_mask, np.float32)
    ent_emb = np.asarray(ent_emb, np.float32)

    if not np.all(attention_mask == 1.0):
        return _reference_numpy(
            hidden, entity_labels, attention_mask, np.asarray(gt_entity),
            np.asarray(gt_head), np.asarray(gt_tail), ent_emb,
            np.asarray(W_ent, np.float32), np.asarray(b_ent, np.float32),
            np.asarray(W_head, np.float32), np.asarray(b_head, np.float32),
            np.asarray(W_tail, np.float32), np.asarray(b_tail, np.float32))

    W_all = np.concatenate(
        [np.asarray(W_ent, np.float32), np.asarray(W_head, np.float32),
         np.asarray(W_tail, np.float32)], axis=0)       # [544, 1088]
    b_all = np.concatenate(
        [np.asarray(b_ent, np.float32), np.asarray(b_head, np.float32),
         np.asarray(b_tail, np.float32)], axis=0)       # [544]
    perm = _build_perm()
    Wp, bp = W_all[perm], b_all[perm]
    wtb = np.zeros((KPAD, MTOT), np.float32)
    wtb[:HID + LAB] = Wp.T
    wtb[HID + LAB] = bp
    wtb = wtb.astype(ml_dtypes.bfloat16)

    trig, jtril = _host_tables()

    in_maps = []
    for b in range(B):
        xT = np.zeros((KPAD, S), np.float32)
        xT[:HID] = hidden[b].T
        xT[HID:HID + LAB] = ent_emb[entity_labels[b]].T
        xT[HID + LAB] = 1.0
        in_maps.append(dict(xT=xT.astype(ml_dtypes.bfloat16), wtb=wtb,
                            trig=trig, jtril=jtril))

    nc = _get_nc()
    res = run_bass_kernel_spmd(nc, in_maps, core_ids=list(range(NCORES)),
                               trace=_want_trace)

    gts = {0: np.asarray(gt_entity), 2: np.asarray(gt_head),
           3: np.asarray(gt_tail)}
    total = 0.0
    for b in range(B):
        out = res.results[b]
        sums = out["sums"].astype(np.float64)      # [128, SUMS_COLS]
        qkv = out["qkout"]                         # [8, 68, 1024]
        for h, (gq, gk, is_tril) in enumerate(_HEADS):
            E = float(np.sum(sums[:, _ACC_OFF[h]:_ACC_OFF[h + 1]]))
            if h < 2:
                gt = gts[0][b, h]
            else:
                gt = gts[h][b, 0]
            total += _mcce_host(E, qkv[gq], qkv[gk], gt)

    if _want_trace:
        kernel._last_results = res
    return np.array(total, dtype=np.float32)



# revision 3
# speedup vs baseline: 1.0503x; 1.0503x over previous
"""Bass/Trainium2 kernel for nn_GPREDecoder (GlobalPointer relation-extraction loss).

Strategy: data-parallel over batch (B=8 -> 8 cores, 1 example per core).
Per example the device computes:
  - projT = W_all @ x_aug.T (bf16, bias folded as an extra contraction row),
    channel layout permuted so the no-rope "head" groups finish first
  - RoPE rotation (J-matmul on PE + cos/sin elementwise on DVE, bf16)
  - per-head S x S logit rounds on PE into [128, 2048] psum tiles,
    exp(SCALE*logit) on ACT with fused per-row accumulation (2048-wide spans)
  - ships per-round exp-sums and the final bf16 q/k tensors
Host gathers the 64 ground-truth pairs per head from q/k and applies the
multilabel-CE pos/neg log corrections in float64.
"""

import ml_dtypes
import numpy as np
from contextlib import ExitStack

import concourse.bass as bass
import concourse.mybir as mybir
import concourse.tile as tile
from concourse import bacc
from concourse.bass_utils import run_bass_kernel_spmd

B, S, HID, LAB = 8, 1024, 1024, 64
HD = 68
SCALE = 1.0 / HD**0.5
INF = 1.0e12
NCORES = 8
NEG_BIG = -1.0e9  # additive pre-scale mask; exp(SCALE*NEG_BIG) == 0 in fp32
KROWS = HID + LAB + 1          # 1089 real contraction rows (bias folded)
NKT = 9
KT_ROWS = [128] * 8 + [KROWS - 8 * 128]  # last kt tile has 65 live rows
KPAD = NKT * 128               # host-side padded allocation

# groups: 0=qe0 1=ke0 2=qe1 3=ke1 4=qh 5=kh 6=qt 7=kt
_GROUP_ORIG = [0, 68, 136, 204, 272, 340, 408, 476]
# projection channel layout: (wtb_col0, group, group_off, cnt)
_LAYOUT = [
    (0, 4, 0, 68), (68, 5, 0, 60),
    (128, 0, 0, 68), (196, 5, 60, 8), (204, 6, 0, 52),
    (256, 1, 0, 68), (324, 6, 52, 16), (340, 7, 0, 44),
    (384, 2, 0, 68), (452, 7, 44, 24),
    (476, 3, 0, 68),
]
T_OFF = [0, 128, 256, 384, 476]   # wtb col offset per proj m-tile
T_W = [128, 128, 128, 92, 68]     # live width per proj m-tile
MTOT = 544

# spill-group assembly: dst_tile -> [(src_tile, src_row0, cnt, dst_row0)]
_ASM = {
    5: [(0, 68, 60, 0), (1, 68, 8, 60)],   # kh
    6: [(1, 76, 52, 0), (2, 68, 16, 52)],  # qt
    7: [(2, 84, 44, 0), (3, 68, 24, 44)],  # kt
}

# full-head rounds: 4 rounds x 2048 span (2 m-tiles each)
_FULL_ROUNDS = [
    (2048,
     [(2 * x, 0, 0, 512), (2 * x, 512, 512, 512),
      (2 * x + 1, 1024, 0, 512), (2 * x + 1, 1536, 512, 512)],
     [])
    for x in range(4)
]
# tril-head rounds: (span, pieces[(m, lo, src, w)], masks[(kind, off)])
# kind 0 = tril (diag block), 1 = all -inf (m7's 128-col pad)
_TRIL_PIECES = [
    (1920, [(0, 0, 0, 1024), (1, 1024, 128, 896)],
     [(0, 0), (0, 1024)]),
    (1664, [(2, 0, 256, 768), (3, 768, 384, 640), (6, 1408, 768, 256)],
     [(0, 0), (0, 768), (0, 1408)]),
    (1152, [(4, 0, 512, 512), (5, 512, 640, 384), (7, 896, 768, 256)],
     [(0, 0), (0, 512), (1, 896), (0, 1024)]),
]


def _chunks(pieces):
    """Split round pieces at 512-col psum bank boundaries."""
    out = []
    for m, lo, src, w in pieces:
        off = 0
        while off < w:
            n = min(512 - ((lo + off) % 512), w - off)
            out.append((m, lo + off, src + off, n))
            off += n
    return out


_TRIL_ROUNDS = [(sp, _chunks(pc), mk) for sp, pc, mk in _TRIL_PIECES]
_FULL_ROUNDS = [(sp, _chunks(pc), mk) for sp, pc, mk in _FULL_ROUNDS]

# head emission order: A=(qh,kh) full, B=(qe0,ke0) tril, C=(qt,kt) full,
# D=(qe1,ke1) tril.  sums column ranges follow emission order.
NSUM = 14


def _build_perm():
    """perm[wtb_col] = original channel index."""
    perm = np.zeros(MTOT, np.int64)
    for col0, g, goff, cnt in _LAYOUT:
        perm[col0:col0 + cnt] = np.arange(
            _GROUP_ORIG[g] + goff, _GROUP_ORIG[g] + goff + cnt)
    return perm


def _build_nc():
    f32 = mybir.dt.float32
    bf16 = mybir.dt.bfloat16
    Exp = mybir.ActivationFunctionType.Exp
    mult = mybir.AluOpType.mult
    add = mybir.AluOpType.add

    nc = bacc.Bacc("TRN2", target_bir_lowering=False)

    xT = nc.dram_tensor("xT", [KPAD, S], bf16, kind="ExternalInput")
    wtb = nc.dram_tensor("wtb", [KPAD, MTOT], bf16, kind="ExternalInput")
    jmat = nc.dram_tensor("jmat", [128, 128], bf16, kind="ExternalInput")
    cosb = nc.dram_tensor("cosb", [HD, S], bf16, kind="ExternalInput")
    sinf = nc.dram_tensor("sinf", [HD, S], f32, kind="ExternalInput")
    masks = nc.dram_tensor("masks", [128, 256], f32, kind="ExternalInput")
    sums = nc.dram_tensor("sums", [128, NSUM], f32, kind="ExternalOutput")
    qkout = nc.dram_tensor("qkout", [8, HD, S], bf16, kind="ExternalOutput")

    xT_r = xT.rearrange("(o p) f -> p o f", p=128)    # [128, 9, 1024]
    wtb_r = wtb.rearrange("(o p) f -> p o f", p=128)  # [128, 9, 544]

    with tile.TileContext(nc) as tc, ExitStack() as ctx:
        singles = ctx.enter_context(tc.tile_pool(name="singles", bufs=1))
        scratch = ctx.enter_context(tc.tile_pool(name="scratch", bufs=2))

        xT_sb = singles.tile([128, NKT, S], bf16, tag="xT_sb", name="xT_sb")
        wtb_sb = singles.tile([128, NKT, MTOT], bf16, tag="wtb_sb",
                              name="wtb_sb")
        jmat_sb = singles.tile([128, 128], bf16, tag="jmat_sb", name="jmat_sb")
        cos_sb = singles.tile([HD, S], bf16, tag="cos_sb", name="cos_sb")
        sin_sb = singles.tile([HD, S], f32, tag="sin_sb", name="sin_sb")
        masks_sb = singles.tile([128, 256], f32, tag="masks_sb",
                                name="masks_sb")
        dense = [singles.tile([128, S], bf16, tag=f"dense{t}",
                              name=f"dense{t}") for t in range(5)]
        asm = {g: singles.tile([HD, S], bf16, tag=f"asm{g}", name=f"asm{g}")
               for g in (5, 6, 7)}
        qrot = {g: singles.tile([HD, S], bf16, tag=f"qrot{g}",
                                name=f"qrot{g}") for g in (0, 1, 2, 3)}
        sums_sb = singles.tile([128, NSUM], f32, tag="sums_sb",
                               name="sums_sb")
        dummy = singles.tile([1, 8], f32, tag="dummy", name="dummy")

        tril_sb = masks_sb[:, 0:128]
        neg_sb = masks_sb[:, 128:256]

        # matmul operand source per group (bf16 [68, S] views)
        def gsrc(g):
            if g == 0:
                return dense[1][0:HD, :]
            if g == 1:
                return dense[2][0:HD, :]
            if g == 2:
                return dense[3][0:HD, :]
            if g == 3:
                return dense[4][0:HD, :]
            if g == 4:
                return dense[0][0:HD, :]
            return asm[g][:, :]

        def gfin(g):  # post-rope operand
            return qrot[g][:, :] if g < 4 else gsrc(g)

        # Early: zero accumulators; pre-warm the ACT exp table load.
        nc.vector.memset(sums_sb[:], 0.0)
        nc.vector.memset(dummy[:], 0.0)
        nc.scalar.activation(dummy[:], dummy[:], Exp)

        # ---- input DMAs: 3 queues, per-kt pacing -------------------------
        # scalar: xT kt 0,2,4,6,8 ; sync: wtb[:, :256] + xT kt 1,3,5,7 ;
        # gpsimd (SWDGE): constants + wtb[:, 256:544]
        def dma_xt(eng, k):
            eng.dma_start(out=xT_sb[0:KT_ROWS[k], k, :],
                          in_=xT_r[0:KT_ROWS[k], k, :])

        def dma_wtb(eng, k0, k1, c0, c1):
            # full 128 rows even for kt8 (host zero-pads) to batch one DMA
            eng.dma_start(out=wtb_sb[:, k0:k1, c0:c1],
                          in_=wtb_r[:, k0:k1, c0:c1])

        dma_xt(nc.scalar, 0)
        dma_wtb(nc.sync, 0, 1, 0, 256)
        nc.gpsimd.dma_start(out=jmat_sb[:], in_=jmat[:, :])
        nc.gpsimd.dma_start(out=cos_sb[:], in_=cosb[:, :])
        nc.gpsimd.dma_start(out=sin_sb[:], in_=sinf[:, :])
        nc.gpsimd.dma_start(out=masks_sb[:], in_=masks[:, :])
        dma_wtb(nc.sync, 1, 2, 0, 256)
        dma_xt(nc.scalar, 2)
        dma_xt(nc.sync, 1)
        dma_wtb(nc.sync, 2, 5, 0, 256)
        dma_xt(nc.scalar, 4)
        dma_xt(nc.sync, 3)
        dma_wtb(nc.sync, 5, 9, 0, 256)
        dma_xt(nc.scalar, 6)
        dma_xt(nc.sync, 5)
        dma_xt(nc.scalar, 8)
        dma_xt(nc.sync, 7)
        dma_wtb(nc.gpsimd, 0, 9, 256, MTOT)

        pool_h = ctx.enter_context(
            tc.tile_pool(name="ph", bufs=1, space="PSUM"))

        acc_col = [0]

        def head_round(pools, q, k, span, chunks, mks, label):
            ph = pools[0].tile([128, 2048], f32, tag=f"ph{pools[1]}",
                               name=label)
            for (m, lo, src, n) in chunks:
                nc.tensor.matmul(
                    ph[:, lo:lo + n],
                    q[:, m * 128:(m + 1) * 128],
                    k[:, src:src + n],
                    start=True, stop=True,
                )
            for kind, off in mks:
                msk = tril_sb if kind == 0 else neg_sb
                nc.vector.tensor_tensor(ph[:, off:off + 128],
                                        ph[:, off:off + 128], msk, add)
            nc.scalar.activation(
                ph[:, 0:span], ph[:, 0:span], Exp, scale=SCALE,
                accum_out=sums_sb[:, acc_col[0]:acc_col[0] + 1])
            acc_col[0] += 1

        with tc.tile_pool(name="pp", bufs=2, space="PSUM") as pool_p:

            def proj_tile(t):
                pt = pool_p.tile([128, S], f32, tag="pp", name=f"proj{t}")
                w = T_W[t]
                for kt in range(NKT):
                    r = KT_ROWS[kt]
                    for c in (0, 512):
                        nc.tensor.matmul(
                            pt[0:w, c:c + 512],
                            wtb_sb[0:r, kt, T_OFF[t]:T_OFF[t] + w],
                            xT_sb[0:r, kt, c:c + 512],
                            start=(kt == 0), stop=(kt == NKT - 1),
                        )
                return pt

            def proj_pair(t0, t1):
                pa = pool_p.tile([128, S], f32, tag="pp", name=f"proj{t0}")
                pb = pool_p.tile([128, S], f32, tag="pp", name=f"proj{t1}")
                for kt in range(NKT):
                    r = KT_ROWS[kt]
                    for t, pt in ((t0, pa), (t1, pb)):
                        w = T_W[t]
                        for c in (0, 512):
                            nc.tensor.matmul(
                                pt[0:w, c:c + 512],
                                wtb_sb[0:r, kt, T_OFF[t]:T_OFF[t] + w],
                                xT_sb[0:r, kt, c:c + 512],
                                start=(kt == 0), stop=(kt == NKT - 1),
                            )
                return pa, pb

            def evac(t, pt):
                w = T_W[t]
                nc.vector.tensor_copy(out=dense[t][0:w, :], in_=pt[0:w, :])

            def assemble(g, eng):
                for (st, r0, cnt, d0) in _ASM[g]:
                    eng.dma_start(out=asm[g][d0:d0 + cnt, :],
                                  in_=dense[st][r0:r0 + cnt, :])

            def jrot(t):
                pj = pool_p.tile([128, S], f32, tag="pp", name=f"j{t}")
                r = T_W[t]
                for c in (0, 512):
                    nc.tensor.matmul(pj[:, c:c + 512], jmat_sb[0:r, :],
                                     dense[t][0:r, c:c + 512],
                                     start=True, stop=True)
                return pj

            def rope(g, pj):
                rtmp = scratch.tile([HD, S], bf16, tag="rtmp",
                                    name=f"rtmp{g}")
                nc.vector.tensor_tensor(rtmp[:, :], pj[0:HD, :], sin_sb,
                                        mult)
                nc.vector.tensor_tensor(qrot[g][:, :], gsrc(g), cos_sb, mult)
                nc.vector.tensor_tensor(qrot[g][:, :], qrot[g][:, :],
                                        rtmp[:, :], add)

            # ---- phase 1: proj tiles 0,1 (qh + kh + qe0 + qt-part) ------
            pa, pb = proj_pair(0, 1)
            evac(0, pa)
            evac(1, pb)
            assemble(5, nc.sync)                       # kh
            nc.gpsimd.dma_start(out=qkout[4], in_=dense[0][0:HD, :])  # qh
            pj0 = jrot(1)                              # J(qe0)
            rope(0, pj0)
            nc.gpsimd.dma_start(out=qkout[5], in_=asm[5][:, :])       # kh

            # ---- head A (qh x kh, full) round 0 -------------------------
            qA, kA = gfin(4), gfin(5)
            sp, ch, mk = _FULL_ROUNDS[0]
            head_round((pool_h, 0), qA, kA, sp, ch, mk, "A0")

            # ---- proj tile 2 (ke0 + qt/kt spill) ------------------------
            pt2 = proj_tile(2)
            evac(2, pt2)
            assemble(6, nc.sync)                       # qt
            nc.gpsimd.dma_start(out=qkout[0], in_=qrot[0][:, :])      # qe0r

            sp, ch, mk = _FULL_ROUNDS[1]
            head_round((pool_h, 0), qA, kA, sp, ch, mk, "A1")

            pj1 = jrot(2)                              # J(ke0)
            rope(1, pj1)
            nc.gpsimd.dma_start(out=qkout[6], in_=asm[6][:, :])       # qt
            nc.gpsimd.dma_start(out=qkout[1], in_=qrot[1][:, :])      # ke0r

            sp, ch, mk = _FULL_ROUNDS[2]
            head_round((pool_h, 0), qA, kA, sp, ch, mk, "A2")

            # ---- proj tile 3 (qe1 + kt spill) ---------------------------
            pt3 = proj_tile(3)
            evac(3, pt3)
            assemble(7, nc.sync)                       # kt
            nc.gpsimd.dma_start(out=qkout[7], in_=asm[7][:, :])       # kt

            sp, ch, mk = _FULL_ROUNDS[3]
            head_round((pool_h, 0), qA, kA, sp, ch, mk, "A3")

            # ---- head B (qe0 x ke0, tril) round 0 + proj tile 4 ---------
            qB, kB = gfin(0), gfin(1)
            sp, ch, mk = _TRIL_ROUNDS[0]
            head_round((pool_h, 0), qB, kB, sp, ch, mk, "B0")

            pt4 = proj_tile(4)
            evac(4, pt4)

            sp, ch, mk = _TRIL_ROUNDS[1]
            head_round((pool_h, 0), qB, kB, sp, ch, mk, "B1")

            pj2 = jrot(3)                              # J(qe1)
            rope(2, pj2)
            nc.gpsimd.dma_start(out=qkout[2], in_=qrot[2][:, :])      # qe1r

            sp, ch, mk = _TRIL_ROUNDS[2]
            head_round((pool_h, 0), qB, kB, sp, ch, mk, "B2")

            pj3 = jrot(4)                              # J(ke1)
            rope(3, pj3)
            nc.gpsimd.dma_start(out=qkout[3], in_=qrot[3][:, :])      # ke1r

        # proj pool released; open a second 2048 pool for double buffering
        with tc.tile_pool(name="ph2", bufs=1, space="PSUM") as pool_h2:
            qC, kC = gfin(6), gfin(7)
            qD, kD = gfin(2), gfin(3)
            tail = ([(qC, kC, r) for r in _FULL_ROUNDS]
                    + [(qD, kD, r) for r in _TRIL_ROUNDS])
            for i, (q, k, (sp, ch, mk)) in enumerate(tail):
                pool = (pool_h, 0) if i % 2 == 0 else (pool_h2, 1)
                head_round(pool, q, k, sp, ch, mk, f"T{i}")

        nc.sync.dma_start(out=sums[:, :], in_=sums_sb[:, :])
        assert acc_col[0] == NSUM

    nc.finalize()
    return nc


_NC_CACHE = None


def _get_nc():
    global _NC_CACHE
    if _NC_CACHE is None:
        _NC_CACHE = _build_nc()
    return _NC_CACHE


def _host_tables():
    pos = np.arange(S, dtype=np.float64)[:, None]
    inv = np.power(10000.0, -2.0 * np.arange(HD // 2, dtype=np.float64) / HD)
    ang = pos * inv                                   # [S, 34]
    cosb = np.repeat(np.cos(ang), 2, axis=1).T.astype(ml_dtypes.bfloat16)
    sinf = np.repeat(np.sin(ang), 2, axis=1).T.astype(np.float32)
    jmat = np.zeros((128, 128), np.float32)
    for i in range(HD // 2):
        # J[2i, 2i+1] = -1 ; J[2i+1, 2i] = +1  -> stored transposed
        jmat[2 * i + 1, 2 * i] = -1.0
        jmat[2 * i, 2 * i + 1] = 1.0
    jmat = jmat.astype(ml_dtypes.bfloat16)
    masks = np.zeros((128, 256), np.float32)
    masks[:, 0:128] = np.where(
        np.arange(128)[None, :] >= np.arange(128)[:, None], 0.0, NEG_BIG)
    masks[:, 128:256] = NEG_BIG
    return jmat, cosb, sinf, masks


def _mcce_host(E_dev, q, k, gt):
    """pos/neg multilabel-CE for one (example, head). q,k: [68,S]; gt: [P,2]."""
    i = gt[:, 0].astype(np.int64)
    j = gt[:, 1].astype(np.int64)
    flat = i * S + j
    lv = np.sum(q[:, i].astype(np.float64) * k[:, j].astype(np.float64),
                axis=0) * SCALE                       # [P]
    live = flat != 0
    pos_loss = np.log1p(np.sum(np.exp(-lv[live])))
    l00 = float(np.sum(q[:, 0].astype(np.float64) * k[:, 0].astype(np.float64))
                * SCALE)
    uf, ui = np.unique(flat, return_index=True)
    keep = uf != 0
    excl = np.exp(l00) + np.sum(np.exp(lv[ui[keep]]))
    neg_loss = np.log1p(E_dev - excl)
    return pos_loss + neg_loss


def _reference_numpy(hidden, entity_labels, attention_mask, gt_entity, gt_head,
                     gt_tail, ent_emb, W_ent, b_ent, W_head, b_head, W_tail,
                     b_tail):
    """Slow exact numpy fallback (used only if attention_mask is not all-ones)."""
    x = np.concatenate([hidden, ent_emb[entity_labels]], axis=-1)

    def rope(v):
        b, s, h, d = v.shape
        pos = np.arange(s, dtype=np.float32)[:, None]
        inv = np.power(10000.0, -2.0 * np.arange(d // 2, dtype=np.float32) / d)
        ang = pos * inv
        sin = np.repeat(np.sin(ang), 2, axis=-1)[None, :, None, :]
        cos = np.repeat(np.cos(ang), 2, axis=-1)[None, :, None, :]
        v2 = np.stack([-v[..., 1::2], v[..., ::2]], axis=-1).reshape(v.shape)
        return v * cos + v2 * sin

    def gp(x, W, b, mask, heads, use_rope, tril):
        bx, sx, _ = x.shape
        proj = (x @ W.T + b).reshape(bx, sx, heads, 2 * HD)
        qw, kw = proj[..., :HD], proj[..., HD:]
        if use_rope:
            qw, kw = rope(qw), rope(kw)
        logits = np.einsum('bmhd,bnhd->bhmn', qw, kw) * SCALE
        pad = mask[:, None, None, :]
        logits = logits * pad - (1.0 - pad) * INF
        if tril:
            logits = logits - np.tril(np.ones((sx, sx), np.float32), -1) * INF
        return logits

    def mcce(y_true, y_pred):
        bx, hx, sx, _ = y_pred.shape
        flat = y_true[..., 0].astype(np.int64) * sx + y_true[..., 1]
        yp = y_pred.reshape(bx, hx, sx * sx).astype(np.float64)
        total = 0.0
        for b in range(bx):
            for h in range(hx):
                f = flat[b, h]
                live = f != 0
                lv = yp[b, h][f]
                pos = np.log1p(np.sum(np.exp(-lv[live])))
                neg_terms = yp[b, h].copy()
                neg_terms[0] = -np.inf
                neg_terms[np.unique(f)] = -np.inf
                neg = np.log1p(np.sum(np.exp(neg_terms)))
                total += pos + neg
        return total

    loss = 0.0
    loss += mcce(gt_entity, gp(x, W_ent, b_ent, attention_mask, 2, True, True))
    loss += mcce(gt_head, gp(x, W_head, b_head, attention_mask, 1, False, False))
    loss += mcce(gt_tail, gp(x, W_tail, b_tail, attention_mask, 1, False, False))
    return np.array(loss, dtype=np.float32)


def kernel(hidden, entity_labels, attention_mask, gt_entity, gt_head, gt_tail,
           ent_emb, W_ent, b_ent, W_head, b_head, W_tail, b_tail,
           _want_trace=False):
    hidden = np.asarray(hidden, np.float32)
    entity_labels = np.asarray(entity_labels)
    attention_mask = np.asarray(attention_mask, np.float32)
    ent_emb = np.asarray(ent_emb, np.float32)

    if not np.all(attention_mask == 1.0):
        return _reference_numpy(
            hidden, entity_labels, attention_mask, np.asarray(gt_entity),
            np.asarray(gt_head), np.asarray(gt_tail), ent_emb,
            np.asarray(W_ent, np.float32), np.asarray(b_ent, np.float32),
            np.asarray(W_head, np.float32), np.asarray(b_head, np.float32),
            np.asarray(W_tail, np.float32), np.asarray(b_tail, np.float32))

    W_all = np.concatenate(
        [np.asarray(W_ent, np.float32), np.asarray(W_head, np.float32),
         np.asarray(W_tail, np.float32)], axis=0)       # [544, 1088]
    b_all = np.concatenate(
        [np.asarray(b_ent, np.float32), np.asarray(b_head, np.float32),
         np.asarray(b_tail, np.float32)], axis=0)       # [544]
    perm = _build_perm()
    Wp, bp = W_all[perm], b_all[perm]
    wtb = np.zeros((KPAD, MTOT), np.float32)
    wtb[:HID + LAB] = Wp.T
    wtb[HID + LAB] = bp
    wtb = wtb.astype(ml_dtypes.bfloat16)

    jmat, cosb, sinf, masks = _host_tables()

    in_maps = []
    for b in range(B):
        xT = np.zeros((KPAD, S), np.float32)
        xT[:HID] = hidden[b].T
        xT[HID:HID + LAB] = ent_emb[entity_labels[b]].T
        xT[HID + LAB] = 1.0
        in_maps.append(dict(xT=xT.astype(ml_dtypes.bfloat16), wtb=wtb,
                            jmat=jmat, cosb=cosb, sinf=sinf, masks=masks))

    nc = _get_nc()
    res = run_bass_kernel_spmd(nc, in_maps, core_ids=list(range(NCORES)),
                               trace=_want_trace)

    # heads in device emission order: A=gt_head, B=ent h0, C=gt_tail, D=ent h1
    heads = [
        (0, 4, (4, 5), "head"),   # sums cols 0:4
        (1, 3, (0, 1), "ent0"),   # cols 4:7
        (2, 4, (6, 7), "tail"),   # cols 7:11
        (3, 3, (2, 3), "ent1"),   # cols 11:14
    ]
    gt_entity = np.asarray(gt_entity)
    gt_head = np.asarray(gt_head)
    gt_tail = np.asarray(gt_tail)
    total = 0.0
    for b in range(B):
        out = res.results[b]
        sums_v = out["sums"].astype(np.float64)    # [128, NSUM]
        qkv = out["qkout"]                         # [8, 68, 1024] bf16
        col = 0
        for _, ncols, (gq, gk), kind in heads:
            E = float(np.sum(sums_v[:, col:col + ncols]))
            col += ncols
            if kind == "head":
                gt = gt_head[b, 0]
            elif kind == "tail":
                gt = gt_tail[b, 0]
            elif kind == "ent0":
                gt = gt_entity[b, 0]
            else:
                gt = gt_entity[b, 1]
            total += _mcce_host(E, qkv[gq], qkv[gk], gt)

    if _want_trace:
        kernel._last_results = res
    return np.array(total, dtype=np.float32)


# revision 7
# speedup vs baseline: 1.1356x; 1.0812x over previous
"""Bass/Trainium2 kernel for nn_GPREDecoder (GlobalPointer relation-extraction loss).

Strategy: data-parallel over batch (B=8 -> 8 cores, 1 example per core).
Per example the device computes:
  - projT = W_all @ x_aug.T (bf16, bias folded as an extra contraction row),
    channel layout permuted so the no-rope "head" groups finish first
  - RoPE rotation (J-matmul on PE + cos/sin elementwise on DVE, bf16)
  - per-head S x S logit rounds on PE into [128, 2048] psum tiles,
    exp(SCALE*logit) on ACT with fused per-row accumulation (2048-wide spans)
  - ships per-round exp-sums and the final bf16 q/k tensors
Host gathers the 64 ground-truth pairs per head from q/k and applies the
multilabel-CE pos/neg log corrections in float64.
"""

import ml_dtypes
import numpy as np
from contextlib import ExitStack

import concourse.bass as bass
import concourse.mybir as mybir
import concourse.tile as tile
from concourse import bacc
from concourse.bass_utils import run_bass_kernel_spmd

B, S, HID, LAB = 8, 1024, 1024, 64
HD = 68
SCALE = 1.0 / HD**0.5
INF = 1.0e12
NCORES = 8
NEG_BIG = -1.0e9  # additive pre-scale mask; exp(SCALE*NEG_BIG) == 0 in fp32
KROWS = HID + LAB + 1          # 1089 real contraction rows (bias folded)
NKT = 9
KT_ROWS = [128] * 8 + [KROWS - 8 * 128]  # last kt tile has 65 live rows
KPAD = NKT * 128               # host-side padded allocation

# groups: 0=qe0 1=ke0 2=qe1 3=ke1 4=qh 5=kh 6=qt 7=kt
_GROUP_ORIG = [0, 68, 136, 204, 272, 340, 408, 476]
# projection channel layout: (wtb_col0, group, group_off, cnt)
_LAYOUT = [
    (0, 4, 0, 68), (68, 5, 0, 60),
    (128, 0, 0, 68), (196, 5, 60, 8), (204, 6, 0, 52),
    (256, 1, 0, 68), (324, 6, 52, 16), (340, 7, 0, 44),
    (384, 2, 0, 68), (452, 7, 44, 24),
    (476, 3, 0, 68),
]
T_OFF = [0, 128, 256, 384, 476]   # wtb col offset per proj m-tile
T_W = [128, 128, 128, 92, 68]     # live width per proj m-tile
MTOT = 544

# spill-group assembly: dst_tile -> [(src_tile, src_row0, cnt, dst_row0)]
_ASM = {
    5: [(0, 68, 60, 0), (1, 68, 8, 60)],   # kh
    6: [(1, 76, 52, 0), (2, 68, 16, 52)],  # qt
    7: [(2, 84, 44, 0), (3, 68, 24, 44)],  # kt
}

# full-head rounds: 4 rounds x 2048 span (2 m-tiles each)
_FULL_ROUNDS = [
    (2048,
     [(2 * x, 0, 0, 512), (2 * x, 512, 512, 512),
      (2 * x + 1, 1024, 0, 512), (2 * x + 1, 1536, 512, 512)],
     [])
    for x in range(4)
]
# tril-head rounds: (span, pieces[(m, lo, src, w)], masks[(kind, off)])
# kind 0 = tril (diag block), 1 = all -inf (m7's 128-col pad)
_TRIL_PIECES = [
    (1920, [(0, 0, 0, 1024), (1, 1024, 128, 896)],
     [(0, 0), (0, 1024)]),
    (1664, [(2, 0, 256, 768), (3, 768, 384, 640), (6, 1408, 768, 256)],
     [(0, 0), (0, 768), (0, 1408)]),
    (1152, [(4, 0, 512, 512), (5, 512, 640, 384), (7, 896, 768, 256)],
     [(0, 0), (0, 512), (1, 896), (0, 1024)]),
]


def _chunks(pieces, head=0):
    """Split round pieces at 512-col psum bank boundaries.

    head>0: emit the first `head` cols of each piece as a separate leading
    chunk (so DVE mask-adds on those blocks overlap the round's remaining
    matmuls)."""
    out = []
    rest = []
    for m, lo, src, w in pieces:
        if head:
            out.append((m, lo, src, head))
            lo, src, w = lo + head, src + head, w - head
        off = 0
        while off < w:
            n = min(512 - ((lo + off) % 512), w - off)
            rest.append((m, lo + off, src + off, n))
            off += n
    return out + rest


_TRIL_ROUNDS = [(sp, _chunks(pc, head=128), mk) for sp, pc, mk in _TRIL_PIECES]
_FULL_ROUNDS = [(sp, _chunks(pc), mk) for sp, pc, mk in _FULL_ROUNDS]

# head emission order: A=(qh,kh) full, B=(qe0,ke0) tril, C=(qt,kt) full,
# D=(qe1,ke1) tril.  sums column ranges follow emission order.
NSUM = 14


def _build_perm():
    """perm[wtb_col] = original channel index."""
    perm = np.zeros(MTOT, np.int64)
    for col0, g, goff, cnt in _LAYOUT:
        perm[col0:col0 + cnt] = np.arange(
            _GROUP_ORIG[g] + goff, _GROUP_ORIG[g] + goff + cnt)
    return perm


def _build_nc():
    f32 = mybir.dt.float32
    bf16 = mybir.dt.bfloat16
    Exp = mybir.ActivationFunctionType.Exp
    mult = mybir.AluOpType.mult
    add = mybir.AluOpType.add

    nc = bacc.Bacc("TRN2", target_bir_lowering=False)

    # host layouts are partition-major so one DMA descriptor covers a
    # partition's whole chunk (descriptor generation is the DMA bottleneck:
    # ~28ns/descriptor per queue)
    xT = nc.dram_tensor("xT", [128, NKT * S], bf16, kind="ExternalInput")
    wtb = nc.dram_tensor("wtb", [128, NKT * MTOT], bf16,
                         kind="ExternalInput")
    jmat = nc.dram_tensor("jmat", [128, 128], bf16, kind="ExternalInput")
    cosb = nc.dram_tensor("cosb", [HD, S], bf16, kind="ExternalInput")
    sinf = nc.dram_tensor("sinf", [HD, S], f32, kind="ExternalInput")
    masks = nc.dram_tensor("masks", [128, 256], f32, kind="ExternalInput")
    sums = nc.dram_tensor("sums", [128, NSUM], f32, kind="ExternalOutput")
    qkout = nc.dram_tensor("qkout", [8, HD, S], bf16, kind="ExternalOutput")

    xT_r = xT.rearrange("p (o f) -> p o f", o=NKT)    # [128, 9, 1024]
    wtb_r = wtb.rearrange("p (o f) -> p o f", o=NKT)  # [128, 9, 544]

    with tile.TileContext(nc) as tc, ExitStack() as ctx:
        singles = ctx.enter_context(tc.tile_pool(name="singles", bufs=1))
        scratch = ctx.enter_context(tc.tile_pool(name="scratch", bufs=2))

        xT_sb = singles.tile([128, NKT, S], bf16, tag="xT_sb", name="xT_sb")
        wtb_sb = singles.tile([128, NKT, MTOT], bf16, tag="wtb_sb",
                              name="wtb_sb")
        jmat_sb = singles.tile([128, 128], bf16, tag="jmat_sb", name="jmat_sb")
        cos_sb = singles.tile([HD, S], bf16, tag="cos_sb", name="cos_sb")
        sin_sb = singles.tile([HD, S], f32, tag="sin_sb", name="sin_sb")
        masks_sb = singles.tile([128, 256], f32, tag="masks_sb",
                                name="masks_sb")
        dense = [singles.tile([128, S], bf16, tag=f"dense{t}",
                              name=f"dense{t}") for t in range(5)]
        asm = {g: singles.tile([HD, S], bf16, tag=f"asm{g}", name=f"asm{g}")
               for g in (5, 6, 7)}
        qrot = {g: singles.tile([HD, S], bf16, tag=f"qrot{g}",
                                name=f"qrot{g}") for g in (0, 1, 2, 3)}
        sums_sb = singles.tile([128, NSUM], f32, tag="sums_sb",
                               name="sums_sb")
        dummy = singles.tile([1, 8], f32, tag="dummy", name="dummy")

        tril_sb = masks_sb[:, 0:128]
        neg_sb = masks_sb[:, 128:256]

        # matmul operand source per group (bf16 [68, S] views)
        def gsrc(g):
            if g == 0:
                return dense[1][0:HD, :]
            if g == 1:
                return dense[2][0:HD, :]
            if g == 2:
                return dense[3][0:HD, :]
            if g == 3:
                return dense[4][0:HD, :]
            if g == 4:
                return dense[0][0:HD, :]
            return asm[g][:, :]

        def gfin(g):  # post-rope operand
            return qrot[g][:, :] if g < 4 else gsrc(g)

        # Early: zero accumulators (scalar prewarm issued after its DMAs).
        nc.vector.memset(sums_sb[:], 0.0)
        nc.vector.memset(dummy[:], 0.0)

        # ---- input DMAs ------------------------------------------------
        # One descriptor per partition per chunk (contiguous in both DRAM
        # and SBUF): chunks of 3 kts = 6KB descriptors.
        def dma_xt(eng, k0, k1):
            eng.dma_start(out=xT_sb[:, k0:k1, :], in_=xT_r[:, k0:k1, :])

        dma_xt(nc.scalar, 0, 3)
        nc.sync.dma_start(out=wtb_sb[:], in_=wtb_r[:, :, :])  # all of wtb
        dma_xt(nc.scalar, 3, 6)
        dma_xt(nc.sync, 6, 9)
        nc.scalar.dma_start(out=sin_sb[:], in_=sinf[:, :])
        nc.scalar.dma_start(out=cos_sb[:], in_=cosb[:, :])
        nc.gpsimd.dma_start(out=jmat_sb[:], in_=jmat[:, :])
        nc.gpsimd.dma_start(out=masks_sb[:], in_=masks[:, :])
        # pre-warm the ACT exp table while input DMAs stream
        nc.scalar.activation(dummy[:], dummy[:], Exp)

        pool_h = ctx.enter_context(
            tc.tile_pool(name="ph", bufs=1, space="PSUM"))

        acc_col = [0]

        def head_round(pools, q, k, span, chunks, mks, label):
            ph = pools[0].tile([128, 2048], f32, tag=f"ph{pools[1]}",
                               name=label)
            for (m, lo, src, n) in chunks:
                nc.tensor.matmul(
                    ph[:, lo:lo + n],
                    q[:, m * 128:(m + 1) * 128],
                    k[:, src:src + n],
                    start=True, stop=True,
                )
            for kind, off in mks:
                msk = tril_sb if kind == 0 else neg_sb
                nc.vector.tensor_tensor(ph[:, off:off + 128],
                                        ph[:, off:off + 128], msk, add)
            nc.scalar.activation(
                ph[:, 0:span], ph[:, 0:span], Exp, scale=SCALE,
                accum_out=sums_sb[:, acc_col[0]:acc_col[0] + 1])
            acc_col[0] += 1

        with tc.tile_pool(name="pp", bufs=2, space="PSUM") as pool_p:

            def proj_tile(t):
                pt = pool_p.tile([128, S], f32, tag="pp", name=f"proj{t}")
                w = T_W[t]
                for kt in range(NKT):
                    r = KT_ROWS[kt]
                    for c in (0, 512):
                        nc.tensor.matmul(
                            pt[0:w, c:c + 512],
                            wtb_sb[0:r, kt, T_OFF[t]:T_OFF[t] + w],
                            xT_sb[0:r, kt, c:c + 512],
                            start=(kt == 0), stop=(kt == NKT - 1),
                        )
                return pt

            def proj_pair(t0, t1):
                pa = pool_p.tile([128, S], f32, tag="pp", name=f"proj{t0}")
                pb = pool_p.tile([128, S], f32, tag="pp", name=f"proj{t1}")
                for kt in range(NKT):
                    r = KT_ROWS[kt]
                    for t, pt in ((t0, pa), (t1, pb)):
                        w = T_W[t]
                        for c in (0, 512):
                            nc.tensor.matmul(
                                pt[0:w, c:c + 512],
                                wtb_sb[0:r, kt, T_OFF[t]:T_OFF[t] + w],
                                xT_sb[0:r, kt, c:c + 512],
                                start=(kt == 0), stop=(kt == NKT - 1),
                            )
                return pa, pb

            def evac(t, pt):
                w = T_W[t]
                nc.vector.tensor_copy(out=dense[t][0:w, :], in_=pt[0:w, :])

            def assemble(g, eng):
                for (st, r0, cnt, d0) in _ASM[g]:
                    eng.dma_start(out=asm[g][d0:d0 + cnt, :],
                                  in_=dense[st][r0:r0 + cnt, :])

            def jrot(t):
                pj = pool_p.tile([128, S], f32, tag="pp", name=f"j{t}")
                r = T_W[t]
                for c in (0, 512):
                    nc.tensor.matmul(pj[:, c:c + 512], jmat_sb[0:r, :],
                                     dense[t][0:r, c:c + 512],
                                     start=True, stop=True)
                return pj

            def rope(g, pj):
                rtmp = scratch.tile([HD, S], bf16, tag="rtmp",
                                    name=f"rtmp{g}")
                nc.vector.tensor_tensor(rtmp[:, :], pj[0:HD, :], sin_sb,
                                        mult)
                nc.vector.tensor_tensor(qrot[g][:, :], gsrc(g), cos_sb, mult)
                nc.vector.tensor_tensor(qrot[g][:, :], qrot[g][:, :],
                                        rtmp[:, :], add)

            # ---- phase 1: proj tiles 0,1 (qh + kh + qe0 + qt-part) ------
            pa, pb = proj_pair(0, 1)
            evac(0, pa)
            evac(1, pb)
            assemble(5, nc.sync)                       # kh
            nc.gpsimd.dma_start(out=qkout[4], in_=dense[0][0:HD, :])  # qh
            pj0 = jrot(1)                              # J(qe0)
            rope(0, pj0)
            nc.gpsimd.dma_start(out=qkout[5], in_=asm[5][:, :])       # kh

            # ---- head A (qh x kh, full) round 0 -------------------------
            qA, kA = gfin(4), gfin(5)
            sp, ch, mk = _FULL_ROUNDS[0]
            head_round((pool_h, 0), qA, kA, sp, ch, mk, "A0")

            # ---- proj tile 2 (ke0 + qt/kt spill) ------------------------
            pt2 = proj_tile(2)
            evac(2, pt2)
            assemble(6, nc.sync)                       # qt
            nc.gpsimd.dma_start(out=qkout[0], in_=qrot[0][:, :])      # qe0r

            sp, ch, mk = _FULL_ROUNDS[1]
            head_round((pool_h, 0), qA, kA, sp, ch, mk, "A1")

            pj1 = jrot(2)                              # J(ke0)
            rope(1, pj1)
            nc.gpsimd.dma_start(out=qkout[6], in_=asm[6][:, :])       # qt
            nc.gpsimd.dma_start(out=qkout[1], in_=qrot[1][:, :])      # ke0r

            sp, ch, mk = _FULL_ROUNDS[2]
            head_round((pool_h, 0), qA, kA, sp, ch, mk, "A2")

            # ---- proj tile 3 (qe1 + kt spill) ---------------------------
            pt3 = proj_tile(3)
            evac(3, pt3)
            assemble(7, nc.sync)                       # kt
            nc.gpsimd.dma_start(out=qkout[7], in_=asm[7][:, :])       # kt

            sp, ch, mk = _FULL_ROUNDS[3]
            head_round((pool_h, 0), qA, kA, sp, ch, mk, "A3")

            # ---- head B (qe0 x ke0, tril) round 0 + proj tile 4 ---------
            qB, kB = gfin(0), gfin(1)
            sp, ch, mk = _TRIL_ROUNDS[0]
            head_round((pool_h, 0), qB, kB, sp, ch, mk, "B0")

            pt4 = proj_tile(4)
            evac(4, pt4)

            sp, ch, mk = _TRIL_ROUNDS[1]
            head_round((pool_h, 0), qB, kB, sp, ch, mk, "B1")

            pj2 = jrot(3)                              # J(qe1)
            rope(2, pj2)
            nc.gpsimd.dma_start(out=qkout[2], in_=qrot[2][:, :])      # qe1r

            sp, ch, mk = _TRIL_ROUNDS[2]
            head_round((pool_h, 0), qB, kB, sp, ch, mk, "B2")

            pj3 = jrot(4)                              # J(ke1)
            rope(3, pj3)
            nc.gpsimd.dma_start(out=qkout[3], in_=qrot[3][:, :])      # ke1r

        # proj pool released; open a second 2048 pool for double buffering
        with tc.tile_pool(name="ph2", bufs=1, space="PSUM") as pool_h2:
            qC, kC = gfin(6), gfin(7)
            qD, kD = gfin(2), gfin(3)
            tail = ([(qC, kC, r) for r in _FULL_ROUNDS]
                    + [(qD, kD, r) for r in _TRIL_ROUNDS])
            for i, (q, k, (sp, ch, mk)) in enumerate(tail):
                pool = (pool_h, 0) if i % 2 == 0 else (pool_h2, 1)
                head_round(pool, q, k, sp, ch, mk, f"T{i}")

        nc.sync.dma_start(out=sums[:, :], in_=sums_sb[:, :])
        assert acc_col[0] == NSUM

    nc.finalize()
    return nc


_NC_CACHE = None


def _get_nc():
    global _NC_CACHE
    if _NC_CACHE is None:
        _NC_CACHE = _build_nc()
    return _NC_CACHE


def _host_tables():
    pos = np.arange(S, dtype=np.float64)[:, None]
    inv = np.power(10000.0, -2.0 * np.arange(HD // 2, dtype=np.float64) / HD)
    ang = pos * inv                                   # [S, 34]
    cosb = np.repeat(np.cos(ang), 2, axis=1).T.astype(ml_dtypes.bfloat16)
    sinf = np.repeat(np.sin(ang), 2, axis=1).T.astype(np.float32)
    jmat = np.zeros((128, 128), np.float32)
    for i in range(HD // 2):
        # J[2i, 2i+1] = -1 ; J[2i+1, 2i] = +1  -> stored transposed
        jmat[2 * i + 1, 2 * i] = -1.0
        jmat[2 * i, 2 * i + 1] = 1.0
    jmat = jmat.astype(ml_dtypes.bfloat16)
    masks = np.zeros((128, 256), np.float32)
    masks[:, 0:128] = np.where(
        np.arange(128)[None, :] >= np.arange(128)[:, None], 0.0, NEG_BIG)
    masks[:, 128:256] = NEG_BIG
    return jmat, cosb, sinf, masks


def _mcce_host(E_dev, q, k, gt):
    """pos/neg multilabel-CE for one (example, head). q,k: [68,S]; gt: [P,2]."""
    i = gt[:, 0].astype(np.int64)
    j = gt[:, 1].astype(np.int64)
    flat = i * S + j
    lv = np.sum(q[:, i].astype(np.float64) * k[:, j].astype(np.float64),
                axis=0) * SCALE                       # [P]
    live = flat != 0
    pos_loss = np.log1p(np.sum(np.exp(-lv[live])))
    l00 = float(np.sum(q[:, 0].astype(np.float64) * k[:, 0].astype(np.float64))
                * SCALE)
    uf, ui = np.unique(flat, return_index=True)
    keep = uf != 0
    excl = np.exp(l00) + np.sum(np.exp(lv[ui[keep]]))
    neg_loss = np.log1p(E_dev - excl)
    return pos_loss + neg_loss


def _reference_numpy(hidden, entity_labels, attention_mask, gt_entity, gt_head,
                     gt_tail, ent_emb, W_ent, b_ent, W_head, b_head, W_tail,
                     b_tail):
    """Slow exact numpy fallback (used only if attention_mask is not all-ones)."""
    x = np.concatenate([hidden, ent_emb[entity_labels]], axis=-1)

    def rope(v):
        b, s, h, d = v.shape
        pos = np.arange(s, dtype=np.float32)[:, None]
        inv = np.power(10000.0, -2.0 * np.arange(d // 2, dtype=np.float32) / d)
        ang = pos * inv
        sin = np.repeat(np.sin(ang), 2, axis=-1)[None, :, None, :]
        cos = np.repeat(np.cos(ang), 2, axis=-1)[None, :, None, :]
        v2 = np.stack([-v[..., 1::2], v[..., ::2]], axis=-1).reshape(v.shape)
        return v * cos + v2 * sin

    def gp(x, W, b, mask, heads, use_rope, tril):
        bx, sx, _ = x.shape
        proj = (x @ W.T + b).reshape(bx, sx, heads, 2 * HD)
        qw, kw = proj[..., :HD], proj[..., HD:]
        if use_rope:
            qw, kw = rope(qw), rope(kw)
        logits = np.einsum('bmhd,bnhd->bhmn', qw, kw) * SCALE
        pad = mask[:, None, None, :]
        logits = logits * pad - (1.0 - pad) * INF
        if tril:
            logits = logits - np.tril(np.ones((sx, sx), np.float32), -1) * INF
        return logits

    def mcce(y_true, y_pred):
        bx, hx, sx, _ = y_pred.shape
        flat = y_true[..., 0].astype(np.int64) * sx + y_true[..., 1]
        yp = y_pred.reshape(bx, hx, sx * sx).astype(np.float64)
        total = 0.0
        for b in range(bx):
            for h in range(hx):
                f = flat[b, h]
                live = f != 0
                lv = yp[b, h][f]
                pos = np.log1p(np.sum(np.exp(-lv[live])))
                neg_terms = yp[b, h].copy()
                neg_terms[0] = -np.inf
                neg_terms[np.unique(f)] = -np.inf
                neg = np.log1p(np.sum(np.exp(neg_terms)))
                total += pos + neg
        return total

    loss = 0.0
    loss += mcce(gt_entity, gp(x, W_ent, b_ent, attention_mask, 2, True, True))
    loss += mcce(gt_head, gp(x, W_head, b_head, attention_mask, 1, False, False))
    loss += mcce(gt_tail, gp(x, W_tail, b_tail, attention_mask, 1, False, False))
    return np.array(loss, dtype=np.float32)


def kernel(hidden, entity_labels, attention_mask, gt_entity, gt_head, gt_tail,
           ent_emb, W_ent, b_ent, W_head, b_head, W_tail, b_tail,
           _want_trace=False):
    hidden = np.asarray(hidden, np.float32)
    entity_labels = np.asarray(entity_labels)
    attention_mask = np.asarray(attention_mask, np.float32)
    ent_emb = np.asarray(ent_emb, np.float32)

    if not np.all(attention_mask == 1.0):
        return _reference_numpy(
            hidden, entity_labels, attention_mask, np.asarray(gt_entity),
            np.asarray(gt_head), np.asarray(gt_tail), ent_emb,
            np.asarray(W_ent, np.float32), np.asarray(b_ent, np.float32),
            np.asarray(W_head, np.float32), np.asarray(b_head, np.float32),
            np.asarray(W_tail, np.float32), np.asarray(b_tail, np.float32))

    W_all = np.concatenate(
        [np.asarray(W_ent, np.float32), np.asarray(W_head, np.float32),
         np.asarray(W_tail, np.float32)], axis=0)       # [544, 1088]
    b_all = np.concatenate(
        [np.asarray(b_ent, np.float32), np.asarray(b_head, np.float32),
         np.asarray(b_tail, np.float32)], axis=0)       # [544]
    perm = _build_perm()
    Wp, bp = W_all[perm], b_all[perm]
    wtb = np.zeros((KPAD, MTOT), np.float32)
    wtb[:HID + LAB] = Wp.T
    wtb[HID + LAB] = bp
    # partition-major: [128, kt, cols] flattened
    wtb = np.ascontiguousarray(
        wtb.reshape(NKT, 128, MTOT).transpose(1, 0, 2).reshape(128, -1)
    ).astype(ml_dtypes.bfloat16)

    jmat, cosb, sinf, masks = _host_tables()

    in_maps = []
    for b in range(B):
        xT = np.zeros((KPAD, S), np.float32)
        xT[:HID] = hidden[b].T
        xT[HID:HID + LAB] = ent_emb[entity_labels[b]].T
        xT[HID + LAB] = 1.0
        xT = np.ascontiguousarray(
            xT.reshape(NKT, 128, S).transpose(1, 0, 2).reshape(128, -1)
        ).astype(ml_dtypes.bfloat16)
        in_maps.append(dict(xT=xT, wtb=wtb,
                            jmat=jmat, cosb=cosb, sinf=sinf, masks=masks))

    nc = _get_nc()
    res = run_bass_kernel_spmd(nc, in_maps, core_ids=list(range(NCORES)),
                               trace=_want_trace)

    # heads in device emission order: A=gt_head, B=ent h0, C=gt_tail, D=ent h1
    heads = [
        (0, 4, (4, 5), "head"),   # sums cols 0:4
        (1, 3, (0, 1), "ent0"),   # cols 4:7
        (2, 4, (6, 7), "tail"),   # cols 7:11
        (3, 3, (2, 3), "ent1"),   # cols 11:14
    ]
    gt_entity = np.asarray(gt_entity)
    gt_head = np.asarray(gt_head)
    gt_tail = np.asarray(gt_tail)
    total = 0.0
    for b in range(B):
        out = res.results[b]
        sums_v = out["sums"].astype(np.float64)    # [128, NSUM]
        qkv = out["qkout"]                         # [8, 68, 1024] bf16
        col = 0
        for _, ncols, (gq, gk), kind in heads:
            E = float(np.sum(sums_v[:, col:col + ncols]))
            col += ncols
            if kind == "head":
                gt = gt_head[b, 0]
            elif kind == "tail":
                gt = gt_tail[b, 0]
            elif kind == "ent0":
                gt = gt_entity[b, 0]
            else:
                gt = gt_entity[b, 1]
            total += _mcce_host(E, qkv[gq], qkv[gk], gt)

    if _want_trace:
        kernel._last_results = res
    return np.array(total, dtype=np.float32)


# revision 15
# speedup vs baseline: 1.1940x; 1.0514x over previous
"""Bass/Trainium2 kernel for nn_GPREDecoder (GlobalPointer relation-extraction loss).

Strategy: data-parallel over batch (B=8 -> 8 cores, 1 example per core).
Per example the device computes:
  - projT = W_all @ x_aug.T (bf16, bias folded as an extra contraction row),
    channel layout permuted so the no-rope "head" groups finish first
  - RoPE rotation (J-matmul on PE + cos/sin elementwise on DVE, bf16)
  - per-head S x S logit rounds on PE into [128, 2048] psum tiles,
    exp(SCALE*logit) on ACT with fused per-row accumulation (2048-wide spans)
  - ships per-round exp-sums and the final bf16 q/k tensors
Host gathers the 64 ground-truth pairs per head from q/k and applies the
multilabel-CE pos/neg log corrections in float64.
"""

import ml_dtypes
import numpy as np
from contextlib import ExitStack

import concourse.bass as bass
import concourse.mybir as mybir
import concourse.tile as tile
from concourse import bacc
from concourse.bass_utils import run_bass_kernel_spmd

B, S, HID, LAB = 8, 1024, 1024, 64
HD = 68
SCALE = 1.0 / HD**0.5
INF = 1.0e12
NCORES = 8
NEG_BIG = -1.0e9  # additive pre-scale mask; exp(SCALE*NEG_BIG) == 0 in fp32
KROWS = HID + LAB + 1          # 1089 real contraction rows (bias folded)
NKT = 9
KT_ROWS = [128] * 8 + [KROWS - 8 * 128]  # last kt tile has 65 live rows
KPAD = NKT * 128               # host-side padded allocation

# groups: 0=qe0 1=ke0 2=qe1 3=ke1 4=qh 5=kh 6=qt 7=kt
_GROUP_ORIG = [0, 68, 136, 204, 272, 340, 408, 476]
# projection channel layout: (wtb_col0, group, group_off, cnt)
_LAYOUT = [
    (0, 4, 0, 68), (68, 5, 0, 60),
    (128, 0, 0, 68), (196, 5, 60, 8), (204, 6, 0, 52),
    (256, 1, 0, 68), (324, 6, 52, 16), (340, 7, 0, 44),
    (384, 2, 0, 68), (452, 7, 44, 24),
    (476, 3, 0, 68),
]
T_OFF = [0, 128, 256, 384, 476]   # wtb col offset per proj m-tile
T_W = [128, 128, 128, 92, 68]     # live width per proj m-tile
MTOT = 544

# spill-group assembly: dst_tile -> [(src_tile, src_row0, cnt, dst_row0)]
_ASM = {
    5: [(0, 68, 60, 0), (1, 68, 8, 60)],   # kh
    6: [(1, 76, 52, 0), (2, 68, 16, 52)],  # qt
    7: [(2, 84, 44, 0), (3, 68, 24, 44)],  # kt
}

# full-head rounds: 4 rounds x 2048 span (2 m-tiles each)
_FULL_ROUNDS = [
    (2048,
     [(2 * x, 0, 0, 512), (2 * x, 512, 512, 512),
      (2 * x + 1, 1024, 0, 512), (2 * x + 1, 1536, 512, 512)],
     [])
    for x in range(4)
]
# tril-head rounds: (span, pieces[(m, lo, src, w)], masks[(kind, off)])
# kind 0 = tril (diag block), 1 = all -inf (m7's 128-col pad)
_TRIL_PIECES = [
    (1920, [(0, 0, 0, 1024), (1, 1024, 128, 896)],
     [(0, 0), (0, 1024)]),
    (1664, [(2, 0, 256, 768), (3, 768, 384, 640), (6, 1408, 768, 256)],
     [(0, 0), (0, 768), (0, 1408)]),
    (1152, [(4, 0, 512, 512), (5, 512, 640, 384), (7, 896, 768, 256)],
     [(0, 0), (0, 512), (1, 896), (0, 1024)]),
]


def _chunks(pieces, head=0):
    """Split round pieces at 512-col psum bank boundaries.

    head>0: emit the first `head` cols of each piece as a separate leading
    chunk (so DVE mask-adds on those blocks overlap the round's remaining
    matmuls)."""
    out = []
    rest = []
    for m, lo, src, w in pieces:
        if head:
            out.append((m, lo, src, head))
            lo, src, w = lo + head, src + head, w - head
        off = 0
        while off < w:
            n = min(512 - ((lo + off) % 512), w - off)
            rest.append((m, lo + off, src + off, n))
            off += n
    return out + rest


_TRIL_ROUNDS = [(sp, _chunks(pc, head=128), mk) for sp, pc, mk in _TRIL_PIECES]
_FULL_ROUNDS = [(sp, _chunks(pc), mk) for sp, pc, mk in _FULL_ROUNDS]

# head emission order: A=(qh,kh) full, B=(qe0,ke0) tril, C=(qt,kt) full,
# D=(qe1,ke1) tril.  sums column ranges follow emission order.
NSUM = 14


def _build_perm():
    """perm[wtb_col] = original channel index."""
    perm = np.zeros(MTOT, np.int64)
    for col0, g, goff, cnt in _LAYOUT:
        perm[col0:col0 + cnt] = np.arange(
            _GROUP_ORIG[g] + goff, _GROUP_ORIG[g] + goff + cnt)
    return perm


def _build_nc():
    f32 = mybir.dt.float32
    bf16 = mybir.dt.bfloat16
    fp8 = mybir.dt.float8e4
    Exp = mybir.ActivationFunctionType.Exp
    mult = mybir.AluOpType.mult
    add = mybir.AluOpType.add

    nc = bacc.Bacc("TRN2", target_bir_lowering=False)

    # host layouts are partition-major (one big DMA descriptor per
    # partition); per-queue DMA streaming is ~95GB/s, so inputs are fp8.
    # wtb is split: A = proj tiles 0-1 (cols 0:256), B = tiles 2-4.
    xT = nc.dram_tensor("xT", [128, NKT * S], fp8, kind="ExternalInput")
    wtbA = nc.dram_tensor("wtbA", [128, NKT * 256], fp8,
                          kind="ExternalInput")
    wtbB = nc.dram_tensor("wtbB", [128, NKT * 288], fp8,
                          kind="ExternalInput")
    jmat = nc.dram_tensor("jmat", [128, 128], bf16, kind="ExternalInput")
    trig = nc.dram_tensor("trig", [HD, 2 * S], bf16, kind="ExternalInput")
    masks = nc.dram_tensor("masks", [128, 256], f32, kind="ExternalInput")
    sums = nc.dram_tensor("sums", [128, NSUM], f32, kind="ExternalOutput")
    qkout = nc.dram_tensor("qkout", [8, HD, S], bf16, kind="ExternalOutput")

    xT_r = xT.rearrange("p (o f) -> p o f", o=NKT)      # [128, 9, 1024]
    wtbA_r = wtbA.rearrange("p (o f) -> p o f", o=NKT)  # [128, 9, 256]
    wtbB_r = wtbB.rearrange("p (o f) -> p o f", o=NKT)  # [128, 9, 288]

    with tile.TileContext(nc) as tc, ExitStack() as ctx:
        singles = ctx.enter_context(tc.tile_pool(name="singles", bufs=1))
        scratch = ctx.enter_context(tc.tile_pool(name="scratch", bufs=2))

        xT_sb = singles.tile([128, NKT, S], fp8, tag="xT_sb", name="xT_sb")
        wtbA_sb = singles.tile([128, NKT, 256], fp8, tag="wtbA_sb",
                               name="wtbA_sb")
        wtbB_sb = singles.tile([128, NKT, 288], fp8, tag="wtbB_sb",
                               name="wtbB_sb")
        jmat_sb = singles.tile([128, 128], bf16, tag="jmat_sb", name="jmat_sb")
        trig_sb = singles.tile([HD, 2 * S], bf16, tag="trig_sb",
                               name="trig_sb")
        masks_sb = singles.tile([128, 256], f32, tag="masks_sb",
                                name="masks_sb")
        dense = [singles.tile([128, S], bf16, tag=f"dense{t}",
                              name=f"dense{t}") for t in range(5)]
        asm = {g: singles.tile([HD, S], bf16, tag=f"asm{g}", name=f"asm{g}")
               for g in (5, 6, 7)}
        qrot = {g: singles.tile([HD, S], bf16, tag=f"qrot{g}",
                                name=f"qrot{g}") for g in (0, 1, 2, 3)}
        sums_sb = singles.tile([128, NSUM], f32, tag="sums_sb",
                               name="sums_sb")
        dummy = singles.tile([1, 8], f32, tag="dummy", name="dummy")

        tril_sb = masks_sb[:, 0:128]
        neg_sb = masks_sb[:, 128:256]
        sin_sb = trig_sb[:, 0:S]
        cos_sb = trig_sb[:, S:2 * S]
        # per proj tile: (wtb sbuf tile, col offset within it)
        wsrc = [(wtbA_sb, 0), (wtbA_sb, 128), (wtbB_sb, 0), (wtbB_sb, 128),
                (wtbB_sb, 220)]

        # matmul operand source per group (bf16 [68, S] views)
        def gsrc(g):
            if g == 0:
                return dense[1][0:HD, :]
            if g == 1:
                return dense[2][0:HD, :]
            if g == 2:
                return dense[3][0:HD, :]
            if g == 3:
                return dense[4][0:HD, :]
            if g == 4:
                return dense[0][0:HD, :]
            return asm[g][:, :]

        def gfin(g):  # post-rope operand
            return qrot[g][:, :] if g < 4 else gsrc(g)

        # Early: zero accumulators (scalar prewarm issued after its DMAs).
        nc.vector.memset(sums_sb[:], 0.0)
        nc.vector.memset(dummy[:], 0.0)

        # ---- input DMAs ------------------------------------------------
        # One descriptor per partition per chunk (contiguous in both DRAM
        # and SBUF).
        def dma_xt(eng, k0, k1):
            eng.dma_start(out=xT_sb[:, k0:k1, :], in_=xT_r[:, k0:k1, :])

        dma_xt(nc.scalar, 0, 3)
        nc.sync.dma_start(out=wtbA_sb[:], in_=wtbA_r[:, :, :])
        dma_xt(nc.scalar, 3, 6)
        dma_xt(nc.sync, 6, 9)
        nc.scalar.dma_start(out=trig_sb[:], in_=trig[:, :])
        nc.sync.dma_start(out=wtbB_sb[:], in_=wtbB_r[:, :, :])
        nc.gpsimd.dma_start(out=jmat_sb[:], in_=jmat[:, :])
        nc.gpsimd.dma_start(out=masks_sb[:], in_=masks[:, :])
        # pre-warm the ACT exp table while input DMAs stream
        nc.scalar.activation(dummy[:], dummy[:], Exp)

        pool_h = ctx.enter_context(
            tc.tile_pool(name="ph", bufs=1, space="PSUM"))

        acc_col = [0]

        def head_round(pools, q, k, span, chunks, mks, label):
            ph = pools[0].tile([128, 2048], f32, tag=f"ph{pools[1]}",
                               name=label)
            for (m, lo, src, n) in chunks:
                nc.tensor.matmul(
                    ph[:, lo:lo + n],
                    q[:, m * 128:(m + 1) * 128],
                    k[:, src:src + n],
                    start=True, stop=True,
                )
            for kind, off in mks:
                msk = tril_sb if kind == 0 else neg_sb
                nc.vector.tensor_tensor(ph[:, off:off + 128],
                                        ph[:, off:off + 128], msk, add)
            nc.scalar.activation(
                ph[:, 0:span], ph[:, 0:span], Exp, scale=SCALE,
                accum_out=sums_sb[:, acc_col[0]:acc_col[0] + 1])
            acc_col[0] += 1

        with tc.tile_pool(name="pp", bufs=2, space="PSUM") as pool_p:

            def proj_tile(t):
                pt = pool_p.tile([128, S], f32, tag="pp", name=f"proj{t}")
                w = T_W[t]
                wsb, woff = wsrc[t]
                for kt in range(NKT):
                    r = KT_ROWS[kt]
                    for c in (0, 512):
                        nc.tensor.matmul(
                            pt[0:w, c:c + 512],
                            wsb[0:r, kt, woff:woff + w],
                            xT_sb[0:r, kt, c:c + 512],
                            start=(kt == 0), stop=(kt == NKT - 1),
                        )
                return pt

            def proj_pair(t0, t1):
                pa = pool_p.tile([128, S], f32, tag="pp", name=f"proj{t0}")
                pb = pool_p.tile([128, S], f32, tag="pp", name=f"proj{t1}")
                for kt in range(NKT):
                    r = KT_ROWS[kt]
                    for t, pt in ((t0, pa), (t1, pb)):
                        w = T_W[t]
                        wsb, woff = wsrc[t]
                        for c in (0, 512):
                            nc.tensor.matmul(
                                pt[0:w, c:c + 512],
                                wsb[0:r, kt, woff:woff + w],
                                xT_sb[0:r, kt, c:c + 512],
                                start=(kt == 0), stop=(kt == NKT - 1),
                            )
                return pa, pb

            def evac(t, pt):
                w = T_W[t]
                nc.vector.tensor_copy(out=dense[t][0:w, :], in_=pt[0:w, :])

            def assemble(g, eng):
                for (st, r0, cnt, d0) in _ASM[g]:
                    eng.dma_start(out=asm[g][d0:d0 + cnt, :],
                                  in_=dense[st][r0:r0 + cnt, :])

            def jrot(t):
                pj = pool_p.tile([128, S], f32, tag="pp", name=f"j{t}")
                r = T_W[t]
                for c in (0, 512):
                    nc.tensor.matmul(pj[:, c:c + 512], jmat_sb[0:r, :],
                                     dense[t][0:r, c:c + 512],
                                     start=True, stop=True)
                return pj

            def rope(g, pj):
                rtmp = scratch.tile([HD, S], bf16, tag="rtmp",
                                    name=f"rtmp{g}")
                nc.vector.tensor_tensor(rtmp[:, :], pj[0:HD, :], sin_sb,
                                        mult)
                nc.vector.tensor_tensor(qrot[g][:, :], gsrc(g), cos_sb, mult)
                nc.vector.tensor_tensor(qrot[g][:, :], qrot[g][:, :],
                                        rtmp[:, :], add)

            # ---- phase 1: proj tiles 0,1 (qh + kh + qe0 + qt-part) ------
            pa, pb = proj_pair(0, 1)
            evac(0, pa)
            evac(1, pb)
            assemble(5, nc.sync)                       # kh
            nc.gpsimd.dma_start(out=qkout[4], in_=dense[0][0:HD, :])  # qh
            pj0 = jrot(1)                              # J(qe0)
            rope(0, pj0)
            nc.gpsimd.dma_start(out=qkout[5], in_=asm[5][:, :])       # kh

            # ---- head A (qh x kh, full) round 0 -------------------------
            qA, kA = gfin(4), gfin(5)
            sp, ch, mk = _FULL_ROUNDS[0]
            head_round((pool_h, 0), qA, kA, sp, ch, mk, "A0")

            # ---- proj tile 2 (ke0 + qt/kt spill) ------------------------
            pt2 = proj_tile(2)
            evac(2, pt2)
            assemble(6, nc.sync)                       # qt
            nc.gpsimd.dma_start(out=qkout[0], in_=qrot[0][:, :])      # qe0r

            sp, ch, mk = _FULL_ROUNDS[1]
            head_round((pool_h, 0), qA, kA, sp, ch, mk, "A1")

            pj1 = jrot(2)                              # J(ke0)
            rope(1, pj1)
            nc.gpsimd.dma_start(out=qkout[6], in_=asm[6][:, :])       # qt
            nc.gpsimd.dma_start(out=qkout[1], in_=qrot[1][:, :])      # ke0r

            sp, ch, mk = _FULL_ROUNDS[2]
            head_round((pool_h, 0), qA, kA, sp, ch, mk, "A2")

            # ---- proj tile 3 (qe1 + kt spill) ---------------------------
            pt3 = proj_tile(3)
            evac(3, pt3)
            assemble(7, nc.sync)                       # kt
            nc.gpsimd.dma_start(out=qkout[7], in_=asm[7][:, :])       # kt

            sp, ch, mk = _FULL_ROUNDS[3]
            head_round((pool_h, 0), qA, kA, sp, ch, mk, "A3")

            # ---- head B (qe0 x ke0, tril) round 0 + proj tile 4 ---------
            qB, kB = gfin(0), gfin(1)
            sp, ch, mk = _TRIL_ROUNDS[0]
            head_round((pool_h, 0), qB, kB, sp, ch, mk, "B0")

            pt4 = proj_tile(4)
            evac(4, pt4)

            sp, ch, mk = _TRIL_ROUNDS[1]
            head_round((pool_h, 0), qB, kB, sp, ch, mk, "B1")

            pj2 = jrot(3)                              # J(qe1)
            rope(2, pj2)
            nc.sync.dma_start(out=qkout[2], in_=qrot[2][:, :])        # qe1r

            sp, ch, mk = _TRIL_ROUNDS[2]
            head_round((pool_h, 0), qB, kB, sp, ch, mk, "B2")

            pj3 = jrot(4)                              # J(ke1)
            rope(3, pj3)
            nc.sync.dma_start(out=qkout[3], in_=qrot[3][:, :])        # ke1r

        # proj pool released; open a second 2048 pool for double buffering
        with tc.tile_pool(name="ph2", bufs=1, space="PSUM") as pool_h2:
            qC, kC = gfin(6), gfin(7)
            qD, kD = gfin(2), gfin(3)
            tail = ([(qC, kC, r) for r in _FULL_ROUNDS]
                    + [(qD, kD, r) for r in _TRIL_ROUNDS])
            for i, (q, k, (sp, ch, mk)) in enumerate(tail):
                pool = (pool_h, 0) if i % 2 == 0 else (pool_h2, 1)
                head_round(pool, q, k, sp, ch, mk, f"T{i}")

        nc.sync.dma_start(out=sums[:, :], in_=sums_sb[:, :])
        assert acc_col[0] == NSUM

    nc.finalize()
    return nc


_NC_CACHE = None


def _get_nc():
    global _NC_CACHE
    if _NC_CACHE is None:
        _NC_CACHE = _build_nc()
    return _NC_CACHE


def _host_tables():
    pos = np.arange(S, dtype=np.float64)[:, None]
    inv = np.power(10000.0, -2.0 * np.arange(HD // 2, dtype=np.float64) / HD)
    ang = pos * inv                                   # [S, 34]
    trig = np.zeros((HD, 2 * S), np.float32)
    trig[:, 0:S] = np.repeat(np.sin(ang), 2, axis=1).T
    trig[:, S:2 * S] = np.repeat(np.cos(ang), 2, axis=1).T
    trig = trig.astype(ml_dtypes.bfloat16)
    jmat = np.zeros((128, 128), np.float32)
    for i in range(HD // 2):
        # J[2i, 2i+1] = -1 ; J[2i+1, 2i] = +1  -> stored transposed
        jmat[2 * i + 1, 2 * i] = -1.0
        jmat[2 * i, 2 * i + 1] = 1.0
    jmat = jmat.astype(ml_dtypes.bfloat16)
    masks = np.zeros((128, 256), np.float32)
    masks[:, 0:128] = np.where(
        np.arange(128)[None, :] >= np.arange(128)[:, None], 0.0, NEG_BIG)
    masks[:, 128:256] = NEG_BIG
    return jmat, trig, masks


def _mcce_host(E_dev, q, k, gt):
    """pos/neg multilabel-CE for one (example, head). q,k: [68,S]; gt: [P,2]."""
    i = gt[:, 0].astype(np.int64)
    j = gt[:, 1].astype(np.int64)
    flat = i * S + j
    lv = np.sum(q[:, i].astype(np.float64) * k[:, j].astype(np.float64),
                axis=0) * SCALE                       # [P]
    live = flat != 0
    pos_loss = np.log1p(np.sum(np.exp(-lv[live])))
    l00 = float(np.sum(q[:, 0].astype(np.float64) * k[:, 0].astype(np.float64))
                * SCALE)
    uf, ui = np.unique(flat, return_index=True)
    keep = uf != 0
    excl = np.exp(l00) + np.sum(np.exp(lv[ui[keep]]))
    neg_loss = np.log1p(E_dev - excl)
    return pos_loss + neg_loss


def _reference_numpy(hidden, entity_labels, attention_mask, gt_entity, gt_head,
                     gt_tail, ent_emb, W_ent, b_ent, W_head, b_head, W_tail,
                     b_tail):
    """Slow exact numpy fallback (used only if attention_mask is not all-ones)."""
    x = np.concatenate([hidden, ent_emb[entity_labels]], axis=-1)

    def rope(v):
        b, s, h, d = v.shape
        pos = np.arange(s, dtype=np.float32)[:, None]
        inv = np.power(10000.0, -2.0 * np.arange(d // 2, dtype=np.float32) / d)
        ang = pos * inv
        sin = np.repeat(np.sin(ang), 2, axis=-1)[None, :, None, :]
        cos = np.repeat(np.cos(ang), 2, axis=-1)[None, :, None, :]
        v2 = np.stack([-v[..., 1::2], v[..., ::2]], axis=-1).reshape(v.shape)
        return v * cos + v2 * sin

    def gp(x, W, b, mask, heads, use_rope, tril):
        bx, sx, _ = x.shape
        proj = (x @ W.T + b).reshape(bx, sx, heads, 2 * HD)
        qw, kw = proj[..., :HD], proj[..., HD:]
        if use_rope:
            qw, kw = rope(qw), rope(kw)
        logits = np.einsum('bmhd,bnhd->bhmn', qw, kw) * SCALE
        pad = mask[:, None, None, :]
        logits = logits * pad - (1.0 - pad) * INF
        if tril:
            logits = logits - np.tril(np.ones((sx, sx), np.float32), -1) * INF
        return logits

    def mcce(y_true, y_pred):
        bx, hx, sx, _ = y_pred.shape
        flat = y_true[..., 0].astype(np.int64) * sx + y_true[..., 1]
        yp = y_pred.reshape(bx, hx, sx * sx).astype(np.float64)
        total = 0.0
        for b in range(bx):
            for h in range(hx):
                f = flat[b, h]
                live = f != 0
                lv = yp[b, h][f]
                pos = np.log1p(np.sum(np.exp(-lv[live])))
                neg_terms = yp[b, h].copy()
                neg_terms[0] = -np.inf
                neg_terms[np.unique(f)] = -np.inf
                neg = np.log1p(np.sum(np.exp(neg_terms)))
                total += pos + neg
        return total

    loss = 0.0
    loss += mcce(gt_entity, gp(x, W_ent, b_ent, attention_mask, 2, True, True))
    loss += mcce(gt_head, gp(x, W_head, b_head, attention_mask, 1, False, False))
    loss += mcce(gt_tail, gp(x, W_tail, b_tail, attention_mask, 1, False, False))
    return np.array(loss, dtype=np.float32)


def kernel(hidden, entity_labels, attention_mask, gt_entity, gt_head, gt_tail,
           ent_emb, W_ent, b_ent, W_head, b_head, W_tail, b_tail,
           _want_trace=False):
    hidden = np.asarray(hidden, np.float32)
    entity_labels = np.asarray(entity_labels)
    attention_mask = np.asarray(attention_mask, np.float32)
    ent_emb = np.asarray(ent_emb, np.float32)

    if not np.all(attention_mask == 1.0):
        return _reference_numpy(
            hidden, entity_labels, attention_mask, np.asarray(gt_entity),
            np.asarray(gt_head), np.asarray(gt_tail), ent_emb,
            np.asarray(W_ent, np.float32), np.asarray(b_ent, np.float32),
            np.asarray(W_head, np.float32), np.asarray(b_head, np.float32),
            np.asarray(W_tail, np.float32), np.asarray(b_tail, np.float32))

    W_all = np.concatenate(
        [np.asarray(W_ent, np.float32), np.asarray(W_head, np.float32),
         np.asarray(W_tail, np.float32)], axis=0)       # [544, 1088]
    b_all = np.concatenate(
        [np.asarray(b_ent, np.float32), np.asarray(b_head, np.float32),
         np.asarray(b_tail, np.float32)], axis=0)       # [544]
    perm = _build_perm()
    Wp, bp = W_all[perm], b_all[perm]
    wtb = np.zeros((KPAD, MTOT), np.float32)
    wtb[:HID + LAB] = Wp.T
    wtb[HID + LAB] = bp
    # partition-major [128, kt, cols], split into tiles 0-1 / 2-4, fp8
    wtb = wtb.reshape(NKT, 128, MTOT).transpose(1, 0, 2)
    wtbA = np.ascontiguousarray(wtb[:, :, 0:256]).reshape(128, -1).astype(
        ml_dtypes.float8_e4m3)
    wtbB = np.ascontiguousarray(wtb[:, :, 256:MTOT]).reshape(128, -1).astype(
        ml_dtypes.float8_e4m3)

    jmat, trig, masks = _host_tables()

    in_maps = []
    for b in range(B):
        xT = np.zeros((KPAD, S), np.float32)
        xT[:HID] = hidden[b].T
        xT[HID:HID + LAB] = ent_emb[entity_labels[b]].T
        xT[HID + LAB] = 1.0
        xT = np.ascontiguousarray(
            xT.reshape(NKT, 128, S).transpose(1, 0, 2).reshape(128, -1)
        ).astype(ml_dtypes.float8_e4m3)
        in_maps.append(dict(xT=xT, wtbA=wtbA, wtbB=wtbB,
                            jmat=jmat, trig=trig, masks=masks))

    nc = _get_nc()
    res = run_bass_kernel_spmd(nc, in_maps, core_ids=list(range(NCORES)),
                               trace=_want_trace)

    # heads in device emission order: A=gt_head, B=ent h0, C=gt_tail, D=ent h1
    heads = [
        (0, 4, (4, 5), "head"),   # sums cols 0:4
        (1, 3, (0, 1), "ent0"),   # cols 4:7
        (2, 4, (6, 7), "tail"),   # cols 7:11
        (3, 3, (2, 3), "ent1"),   # cols 11:14
    ]
    gt_entity = np.asarray(gt_entity)
    gt_head = np.asarray(gt_head)
    gt_tail = np.asarray(gt_tail)
    total = 0.0
    for b in range(B):
        out = res.results[b]
        sums_v = out["sums"].astype(np.float64)    # [128, NSUM]
        qkv = out["qkout"]                         # [8, 68, 1024] bf16
        col = 0
        for _, ncols, (gq, gk), kind in heads:
            E = float(np.sum(sums_v[:, col:col + ncols]))
            col += ncols
            if kind == "head":
                gt = gt_head[b, 0]
            elif kind == "tail":
                gt = gt_tail[b, 0]
            elif kind == "ent0":
                gt = gt_entity[b, 0]
            else:
                gt = gt_entity[b, 1]
            total += _mcce_host(E, qkv[gq], qkv[gk], gt)

    if _want_trace:
        kernel._last_results = res
    return np.array(total, dtype=np.float32)


# revision 21
# speedup vs baseline: 1.2047x; 1.0090x over previous
"""Bass/Trainium2 kernel for nn_GPREDecoder (GlobalPointer relation-extraction loss).

Strategy: data-parallel over batch (B=8 -> 8 cores, 1 example per core).
Per example the device computes:
  - projT = W_all @ x_aug.T (bf16, bias folded as an extra contraction row),
    channel layout permuted so the no-rope "head" groups finish first
  - RoPE rotation (J-matmul on PE + cos/sin elementwise on DVE, bf16)
  - per-head S x S logit rounds on PE into [128, 2048] psum tiles,
    exp(SCALE*logit) on ACT with fused per-row accumulation (2048-wide spans)
  - ships per-round exp-sums and the final bf16 q/k tensors
Host gathers the 64 ground-truth pairs per head from q/k and applies the
multilabel-CE pos/neg log corrections in float64.
"""

import ml_dtypes
import numpy as np
from contextlib import ExitStack

import concourse.bass as bass
import concourse.mybir as mybir
import concourse.tile as tile
from concourse import bacc
from concourse.bass_utils import run_bass_kernel_spmd

B, S, HID, LAB = 8, 1024, 1024, 64
HD = 68
SCALE = 1.0 / HD**0.5
INF = 1.0e12
NCORES = 8
NEG_BIG = -1.0e9  # additive pre-scale mask; exp(SCALE*NEG_BIG) == 0 in fp32
KROWS = HID + LAB + 1          # 1089 real contraction rows (bias folded)
NKT = 9
KT_ROWS = [128] * 8 + [KROWS - 8 * 128]  # last kt tile has 65 live rows
KPAD = NKT * 128               # host-side padded allocation

# groups: 0=qe0 1=ke0 2=qe1 3=ke1 4=qh 5=kh 6=qt 7=kt
_GROUP_ORIG = [0, 68, 136, 204, 272, 340, 408, 476]
# projection channel layout: (wtb_col0, group, group_off, cnt)
# qh and kh are whole in tiles 0/1 so head A needs no spill assembly;
# qe0 (rope, off the early critical path) is the split group instead.
_LAYOUT = [
    (0, 4, 0, 68), (68, 0, 0, 60),
    (128, 5, 0, 68), (196, 0, 60, 8), (204, 6, 0, 52),
    (256, 1, 0, 68), (324, 6, 52, 16), (340, 7, 0, 44),
    (384, 2, 0, 68), (452, 7, 44, 24),
    (476, 3, 0, 68),
]
T_OFF = [0, 128, 256, 384, 476]   # wtb col offset per proj m-tile
T_W = [128, 128, 128, 92, 68]     # live width per proj m-tile
MTOT = 544

# spill-group assembly: dst_tile -> [(src_tile, src_row0, cnt, dst_row0)]
_ASM = {
    0: [(0, 68, 60, 0), (1, 68, 8, 60)],   # qe0
    6: [(1, 76, 52, 0), (2, 68, 16, 52)],  # qt
    7: [(2, 84, 44, 0), (3, 68, 24, 44)],  # kt
}

# full-head rounds: 4 rounds x 2048 span (2 m-tiles each)
_FULL_ROUNDS = [
    (2048,
     [(2 * x, 0, 0, 512), (2 * x, 512, 512, 512),
      (2 * x + 1, 1024, 0, 512), (2 * x + 1, 1536, 512, 512)],
     [])
    for x in range(4)
]
# tril-head rounds: (span, pieces[(m, lo, src, w)], masks[(kind, off)])
# kind 0 = tril (diag block), 1 = all -inf (m7's 128-col pad)
_TRIL_PIECES = [
    (1920, [(0, 0, 0, 1024), (1, 1024, 128, 896)],
     [(0, 0), (0, 1024)]),
    (1664, [(2, 0, 256, 768), (3, 768, 384, 640), (6, 1408, 768, 256)],
     [(0, 0), (0, 768), (0, 1408)]),
    (1152, [(4, 0, 512, 512), (5, 512, 640, 384), (7, 896, 768, 256)],
     [(0, 0), (0, 512), (1, 896), (0, 1024)]),
]


def _chunks(pieces, head=0):
    """Split round pieces at 512-col psum bank boundaries.

    head>0: emit the first `head` cols of each piece as a separate leading
    chunk (so DVE mask-adds on those blocks overlap the round's remaining
    matmuls)."""
    out = []
    rest = []
    for m, lo, src, w in pieces:
        if head:
            out.append((m, lo, src, head))
            lo, src, w = lo + head, src + head, w - head
        off = 0
        while off < w:
            n = min(512 - ((lo + off) % 512), w - off)
            rest.append((m, lo + off, src + off, n))
            off += n
    return out + rest


_TRIL_ROUNDS = [(sp, _chunks(pc, head=128), mk) for sp, pc, mk in _TRIL_PIECES]
_FULL_ROUNDS = [(sp, _chunks(pc), mk) for sp, pc, mk in _FULL_ROUNDS]

# head emission order: A=(qh,kh) full, B=(qe0,ke0) tril, C=(qt,kt) full,
# D=(qe1,ke1) tril.  sums column ranges follow emission order.
NSUM = 14


def _build_perm():
    """perm[wtb_col] = original channel index."""
    perm = np.zeros(MTOT, np.int64)
    for col0, g, goff, cnt in _LAYOUT:
        perm[col0:col0 + cnt] = np.arange(
            _GROUP_ORIG[g] + goff, _GROUP_ORIG[g] + goff + cnt)
    return perm


def _build_nc():
    f32 = mybir.dt.float32
    bf16 = mybir.dt.bfloat16
    fp8 = mybir.dt.float8e4
    Exp = mybir.ActivationFunctionType.Exp
    mult = mybir.AluOpType.mult
    add = mybir.AluOpType.add

    nc = bacc.Bacc("TRN2", target_bir_lowering=False)

    # host layouts are partition-major (one big DMA descriptor per
    # partition); per-queue DMA streaming is ~95GB/s, so inputs are fp8.
    # wtb is split: A = proj tiles 0-1 (cols 0:256), B = tiles 2-4.
    xT = nc.dram_tensor("xT", [128, NKT * S], fp8, kind="ExternalInput")
    wtbA = nc.dram_tensor("wtbA", [128, NKT * 256], fp8,
                          kind="ExternalInput")
    wtbB = nc.dram_tensor("wtbB", [128, NKT * 288], fp8,
                          kind="ExternalInput")
    jmat = nc.dram_tensor("jmat", [128, 128], bf16, kind="ExternalInput")
    trig = nc.dram_tensor("trig", [HD, 2 * S], bf16, kind="ExternalInput")
    masks = nc.dram_tensor("masks", [128, 256], f32, kind="ExternalInput")
    sums = nc.dram_tensor("sums", [128, NSUM], f32, kind="ExternalOutput")
    qkout = nc.dram_tensor("qkout", [8, HD, S], bf16, kind="ExternalOutput")

    xT_r = xT.rearrange("p (o f) -> p o f", o=NKT)      # [128, 9, 1024]
    wtbA_r = wtbA.rearrange("p (o f) -> p o f", o=NKT)  # [128, 9, 256]
    wtbB_r = wtbB.rearrange("p (o f) -> p o f", o=NKT)  # [128, 9, 288]

    with tile.TileContext(nc) as tc, ExitStack() as ctx:
        singles = ctx.enter_context(tc.tile_pool(name="singles", bufs=1))
        scratch = ctx.enter_context(tc.tile_pool(name="scratch", bufs=2))

        xT_sb = singles.tile([128, NKT, S], fp8, tag="xT_sb", name="xT_sb")
        wtbA_sb = singles.tile([128, NKT, 256], fp8, tag="wtbA_sb",
                               name="wtbA_sb")
        wtbB_sb = singles.tile([128, NKT, 288], fp8, tag="wtbB_sb",
                               name="wtbB_sb")
        jmat_sb = singles.tile([128, 128], bf16, tag="jmat_sb", name="jmat_sb")
        trig_sb = singles.tile([HD, 2 * S], bf16, tag="trig_sb",
                               name="trig_sb")
        masks_sb = singles.tile([128, 256], f32, tag="masks_sb",
                                name="masks_sb")
        dense = [singles.tile([128, S], bf16, tag=f"dense{t}",
                              name=f"dense{t}") for t in range(5)]
        asm = {g: singles.tile([HD, S], bf16, tag=f"asm{g}", name=f"asm{g}")
               for g in (0, 6, 7)}
        warm_sb = singles.tile([128, 512], bf16, tag="warm", name="warm")
        qrot = {g: singles.tile([HD, S], bf16, tag=f"qrot{g}",
                                name=f"qrot{g}") for g in (0, 1, 2, 3)}
        sums_sb = singles.tile([128, NSUM], f32, tag="sums_sb",
                               name="sums_sb")
        dummy = singles.tile([1, 8], f32, tag="dummy", name="dummy")

        tril_sb = masks_sb[:, 0:128]
        neg_sb = masks_sb[:, 128:256]
        sin_sb = trig_sb[:, 0:S]
        cos_sb = trig_sb[:, S:2 * S]
        # per proj tile: (wtb sbuf tile, col offset within it)
        wsrc = [(wtbA_sb, 0), (wtbA_sb, 128), (wtbB_sb, 0), (wtbB_sb, 128),
                (wtbB_sb, 220)]

        # matmul operand source per group (bf16 [68, S] views)
        def gsrc(g):
            if g == 1:
                return dense[2][0:HD, :]
            if g == 2:
                return dense[3][0:HD, :]
            if g == 3:
                return dense[4][0:HD, :]
            if g == 4:
                return dense[0][0:HD, :]
            if g == 5:
                return dense[1][0:HD, :]
            return asm[g][:, :]

        def gfin(g):  # post-rope operand
            return qrot[g][:, :] if g < 4 else gsrc(g)

        # Early: zero accumulators (scalar prewarm issued after its DMAs).
        nc.vector.memset(sums_sb[:], 0.0)
        nc.vector.memset(dummy[:], 0.0)
        nc.vector.memset(warm_sb[:], 0.0)

        # ---- input DMAs ------------------------------------------------
        # One descriptor per partition per chunk (contiguous in both DRAM
        # and SBUF).
        def dma_xt(eng, k0, k1):
            eng.dma_start(out=xT_sb[:, k0:k1, :], in_=xT_r[:, k0:k1, :])

        dma_xt(nc.scalar, 0, 3)
        nc.sync.dma_start(out=wtbA_sb[:], in_=wtbA_r[:, :, :])
        nc.gpsimd.dma_start(out=xT_sb[0:KT_ROWS[8], 8, :],
                            in_=xT_r[0:KT_ROWS[8], 8, :])
        dma_xt(nc.scalar, 3, 5)
        dma_xt(nc.sync, 5, 8)
        nc.scalar.dma_start(out=trig_sb[:], in_=trig[:, :])
        nc.sync.dma_start(out=wtbB_sb[:], in_=wtbB_r[:, :, :])
        nc.gpsimd.dma_start(out=jmat_sb[:], in_=jmat[:, :])
        nc.gpsimd.dma_start(out=masks_sb[:], in_=masks[:, :])
        # pre-warm the ACT exp table while input DMAs stream
        nc.scalar.activation(dummy[:], dummy[:], Exp)

        pool_h = ctx.enter_context(
            tc.tile_pool(name="ph", bufs=1, space="PSUM"))

        acc_col = [0]

        def head_round(pools, q, k, span, chunks, mks, label):
            ph = pools[0].tile([128, 2048], f32, tag=f"ph{pools[1]}",
                               name=label)
            for (m, lo, src, n) in chunks:
                nc.tensor.matmul(
                    ph[:, lo:lo + n],
                    q[:, m * 128:(m + 1) * 128],
                    k[:, src:src + n],
                    start=True, stop=True,
                )
            for kind, off in mks:
                msk = tril_sb if kind == 0 else neg_sb
                nc.vector.tensor_tensor(ph[:, off:off + 128],
                                        ph[:, off:off + 128], msk, add)
            nc.scalar.activation(
                ph[:, 0:span], ph[:, 0:span], Exp, scale=SCALE,
                accum_out=sums_sb[:, acc_col[0]:acc_col[0] + 1])
            acc_col[0] += 1

        with tc.tile_pool(name="pp", bufs=2, space="PSUM") as pool_p:

            def proj_tile(t):
                pt = pool_p.tile([128, S], f32, tag="pp", name=f"proj{t}")
                w = T_W[t]
                wsb, woff = wsrc[t]
                for kt in range(NKT):
                    r = KT_ROWS[kt]
                    for c in (0, 512):
                        nc.tensor.matmul(
                            pt[0:w, c:c + 512],
                            wsb[0:r, kt, woff:woff + w],
                            xT_sb[0:r, kt, c:c + 512],
                            start=(kt == 0), stop=(kt == NKT - 1),
                        )
                return pt

            def proj_pair(t0, t1):
                pa = pool_p.tile([128, S], f32, tag="pp", name=f"proj{t0}")
                pb = pool_p.tile([128, S], f32, tag="pp", name=f"proj{t1}")
                for kt in range(NKT):
                    r = KT_ROWS[kt]
                    for t, pt in ((t0, pa), (t1, pb)):
                        w = T_W[t]
                        wsb, woff = wsrc[t]
                        for c in (0, 512):
                            nc.tensor.matmul(
                                pt[0:w, c:c + 512],
                                wsb[0:r, kt, woff:woff + w],
                                xT_sb[0:r, kt, c:c + 512],
                                start=(kt == 0), stop=(kt == NKT - 1),
                            )
                return pa, pb

            def evac(t, pt, eng="dve"):
                w = T_W[t]
                if eng == "act":
                    nc.scalar.copy(out=dense[t][0:w, :], in_=pt[0:w, :])
                else:
                    nc.vector.tensor_copy(out=dense[t][0:w, :],
                                          in_=pt[0:w, :])

            def assemble(g, eng):
                for (st, r0, cnt, d0) in _ASM[g]:
                    eng.dma_start(out=asm[g][d0:d0 + cnt, :],
                                  in_=dense[st][r0:r0 + cnt, :])

            def jrot(src, r):
                pj = pool_p.tile([128, S], f32, tag="pp", name="j")
                for c in (0, 512):
                    nc.tensor.matmul(pj[:, c:c + 512], jmat_sb[0:r, :],
                                     src[0:r, c:c + 512],
                                     start=True, stop=True)
                return pj

            def rope(g, pj):
                rtmp = scratch.tile([HD, S], bf16, tag="rtmp",
                                    name=f"rtmp{g}")
                nc.vector.tensor_tensor(rtmp[:, :], pj[0:HD, :], sin_sb,
                                        mult)
                nc.vector.tensor_tensor(qrot[g][:, :], gsrc(g), cos_sb, mult)
                nc.vector.tensor_tensor(qrot[g][:, :], qrot[g][:, :],
                                        rtmp[:, :], add)

            # ---- PE warm-up: dummy matmuls during the input DMA wait ----
            # gets the HAM clock gate to 8/8 (~3.4us busy) so the real
            # projection streams at 2.4GHz from its first matmul
            pw = pool_p.tile([128, S], f32, tag="pp", name="warmup")
            for _ in range(12):
                nc.tensor.matmul(pw[:, 0:512], warm_sb[:, 0:128],
                                 warm_sb[:, 0:512], start=True, stop=True)

            # ---- phase 1: proj tiles 0,1 (qh | kh + qe0/qt spill) -------
            pa, pb = proj_pair(0, 1)
            evac(0, pa, "act")
            evac(1, pb)
            assemble(0, nc.sync)                       # qe0
            nc.gpsimd.dma_start(out=qkout[4], in_=dense[0][0:HD, :])  # qh
            nc.gpsimd.dma_start(out=qkout[5], in_=dense[1][0:HD, :])  # kh

            # ---- head A (qh x kh, full) round 0 -------------------------
            qA, kA = gfin(4), gfin(5)
            sp, ch, mk = _FULL_ROUNDS[0]
            head_round((pool_h, 0), qA, kA, sp, ch, mk, "A0")
            pj0 = jrot(asm[0], HD)                     # J(qe0)
            rope(0, pj0)

            # ---- proj tile 2 (ke0 + qt/kt spill) ------------------------
            pt2 = proj_tile(2)
            evac(2, pt2)
            assemble(6, nc.sync)                       # qt
            nc.gpsimd.dma_start(out=qkout[0], in_=qrot[0][:, :])      # qe0r
            nc.gpsimd.dma_start(out=qkout[6], in_=asm[6][:, :])       # qt

            sp, ch, mk = _FULL_ROUNDS[1]
            head_round((pool_h, 0), qA, kA, sp, ch, mk, "A1")

            pj1 = jrot(dense[2], HD)                   # J(ke0)
            rope(1, pj1)
            nc.gpsimd.dma_start(out=qkout[1], in_=qrot[1][:, :])      # ke0r

            sp, ch, mk = _FULL_ROUNDS[2]
            head_round((pool_h, 0), qA, kA, sp, ch, mk, "A2")

            # ---- proj tile 3 (qe1 + kt spill) ---------------------------
            pt3 = proj_tile(3)
            evac(3, pt3)
            assemble(7, nc.sync)                       # kt
            nc.gpsimd.dma_start(out=qkout[7], in_=asm[7][:, :])       # kt

            sp, ch, mk = _FULL_ROUNDS[3]
            head_round((pool_h, 0), qA, kA, sp, ch, mk, "A3")

            # ---- head B (qe0 x ke0, tril) round 0 + proj tile 4 ---------
            qB, kB = gfin(0), gfin(1)
            sp, ch, mk = _TRIL_ROUNDS[0]
            head_round((pool_h, 0), qB, kB, sp, ch, mk, "B0")

            pt4 = proj_tile(4)
            evac(4, pt4)

            sp, ch, mk = _TRIL_ROUNDS[1]
            head_round((pool_h, 0), qB, kB, sp, ch, mk, "B1")

            pj2 = jrot(dense[3], HD)                   # J(qe1)
            rope(2, pj2)
            nc.sync.dma_start(out=qkout[2], in_=qrot[2][:, :])        # qe1r

            sp, ch, mk = _TRIL_ROUNDS[2]
            head_round((pool_h, 0), qB, kB, sp, ch, mk, "B2")

            pj3 = jrot(dense[4], HD)                   # J(ke1)
            rope(3, pj3)
            nc.sync.dma_start(out=qkout[3], in_=qrot[3][:, :])        # ke1r

        # proj pool released; open a second 2048 pool for double buffering
        with tc.tile_pool(name="ph2", bufs=1, space="PSUM") as pool_h2:
            qC, kC = gfin(6), gfin(7)
            qD, kD = gfin(2), gfin(3)
            tail = ([(qC, kC, r) for r in _FULL_ROUNDS]
                    + [(qD, kD, r) for r in _TRIL_ROUNDS])
            for i, (q, k, (sp, ch, mk)) in enumerate(tail):
                pool = (pool_h, 0) if i % 2 == 0 else (pool_h2, 1)
                head_round(pool, q, k, sp, ch, mk, f"T{i}")

        nc.sync.dma_start(out=sums[:, :], in_=sums_sb[:, :])
        assert acc_col[0] == NSUM

    nc.finalize()
    return nc


_NC_CACHE = None


def _get_nc():
    global _NC_CACHE
    if _NC_CACHE is None:
        _NC_CACHE = _build_nc()
    return _NC_CACHE


def _host_tables():
    pos = np.arange(S, dtype=np.float64)[:, None]
    inv = np.power(10000.0, -2.0 * np.arange(HD // 2, dtype=np.float64) / HD)
    ang = pos * inv                                   # [S, 34]
    trig = np.zeros((HD, 2 * S), np.float32)
    trig[:, 0:S] = np.repeat(np.sin(ang), 2, axis=1).T
    trig[:, S:2 * S] = np.repeat(np.cos(ang), 2, axis=1).T
    trig = trig.astype(ml_dtypes.bfloat16)
    jmat = np.zeros((128, 128), np.float32)
    for i in range(HD // 2):
        # J[2i, 2i+1] = -1 ; J[2i+1, 2i] = +1  -> stored transposed
        jmat[2 * i + 1, 2 * i] = -1.0
        jmat[2 * i, 2 * i + 1] = 1.0
    jmat = jmat.astype(ml_dtypes.bfloat16)
    masks = np.zeros((128, 256), np.float32)
    masks[:, 0:128] = np.where(
        np.arange(128)[None, :] >= np.arange(128)[:, None], 0.0, NEG_BIG)
    masks[:, 128:256] = NEG_BIG
    return jmat, trig, masks


def _mcce_host(E_dev, q, k, gt):
    """pos/neg multilabel-CE for one (example, head). q,k: [68,S]; gt: [P,2]."""
    i = gt[:, 0].astype(np.int64)
    j = gt[:, 1].astype(np.int64)
    flat = i * S + j
    lv = np.sum(q[:, i].astype(np.float64) * k[:, j].astype(np.float64),
                axis=0) * SCALE                       # [P]
    live = flat != 0
    pos_loss = np.log1p(np.sum(np.exp(-lv[live])))
    l00 = float(np.sum(q[:, 0].astype(np.float64) * k[:, 0].astype(np.float64))
                * SCALE)
    uf, ui = np.unique(flat, return_index=True)
    keep = uf != 0
    excl = np.exp(l00) + np.sum(np.exp(lv[ui[keep]]))
    neg_loss = np.log1p(E_dev - excl)
    return pos_loss + neg_loss


def _reference_numpy(hidden, entity_labels, attention_mask, gt_entity, gt_head,
                     gt_tail, ent_emb, W_ent, b_ent, W_head, b_head, W_tail,
                     b_tail):
    """Slow exact numpy fallback (used only if attention_mask is not all-ones)."""
    x = np.concatenate([hidden, ent_emb[entity_labels]], axis=-1)

    def rope(v):
        b, s, h, d = v.shape
        pos = np.arange(s, dtype=np.float32)[:, None]
        inv = np.power(10000.0, -2.0 * np.arange(d // 2, dtype=np.float32) / d)
        ang = pos * inv
        sin = np.repeat(np.sin(ang), 2, axis=-1)[None, :, None, :]
        cos = np.repeat(np.cos(ang), 2, axis=-1)[None, :, None, :]
        v2 = np.stack([-v[..., 1::2], v[..., ::2]], axis=-1).reshape(v.shape)
        return v * cos + v2 * sin

    def gp(x, W, b, mask, heads, use_rope, tril):
        bx, sx, _ = x.shape
        proj = (x @ W.T + b).reshape(bx, sx, heads, 2 * HD)
        qw, kw = proj[..., :HD], proj[..., HD:]
        if use_rope:
            qw, kw = rope(qw), rope(kw)
        logits = np.einsum('bmhd,bnhd->bhmn', qw, kw) * SCALE
        pad = mask[:, None, None, :]
        logits = logits * pad - (1.0 - pad) * INF
        if tril:
            logits = logits - np.tril(np.ones((sx, sx), np.float32), -1) * INF
        return logits

    def mcce(y_true, y_pred):
        bx, hx, sx, _ = y_pred.shape
        flat = y_true[..., 0].astype(np.int64) * sx + y_true[..., 1]
        yp = y_pred.reshape(bx, hx, sx * sx).astype(np.float64)
        total = 0.0
        for b in range(bx):
            for h in range(hx):
                f = flat[b, h]
                live = f != 0
                lv = yp[b, h][f]
                pos = np.log1p(np.sum(np.exp(-lv[live])))
                neg_terms = yp[b, h].copy()
                neg_terms[0] = -np.inf
                neg_terms[np.unique(f)] = -np.inf
                neg = np.log1p(np.sum(np.exp(neg_terms)))
                total += pos + neg
        return total

    loss = 0.0
    loss += mcce(gt_entity, gp(x, W_ent, b_ent, attention_mask, 2, True, True))
    loss += mcce(gt_head, gp(x, W_head, b_head, attention_mask, 1, False, False))
    loss += mcce(gt_tail, gp(x, W_tail, b_tail, attention_mask, 1, False, False))
    return np.array(loss, dtype=np.float32)


def kernel(hidden, entity_labels, attention_mask, gt_entity, gt_head, gt_tail,
           ent_emb, W_ent, b_ent, W_head, b_head, W_tail, b_tail,
           _want_trace=False):
    hidden = np.asarray(hidden, np.float32)
    entity_labels = np.asarray(entity_labels)
    attention_mask = np.asarray(attention_mask, np.float32)
    ent_emb = np.asarray(ent_emb, np.float32)

    if not np.all(attention_mask == 1.0):
        return _reference_numpy(
            hidden, entity_labels, attention_mask, np.asarray(gt_entity),
            np.asarray(gt_head), np.asarray(gt_tail), ent_emb,
            np.asarray(W_ent, np.float32), np.asarray(b_ent, np.float32),
            np.asarray(W_head, np.float32), np.asarray(b_head, np.float32),
            np.asarray(W_tail, np.float32), np.asarray(b_tail, np.float32))

    W_all = np.concatenate(
        [np.asarray(W_ent, np.float32), np.asarray(W_head, np.float32),
         np.asarray(W_tail, np.float32)], axis=0)       # [544, 1088]
    b_all = np.concatenate(
        [np.asarray(b_ent, np.float32), np.asarray(b_head, np.float32),
         np.asarray(b_tail, np.float32)], axis=0)       # [544]
    perm = _build_perm()
    Wp, bp = W_all[perm], b_all[perm]
    wtb = np.zeros((KPAD, MTOT), np.float32)
    wtb[:HID + LAB] = Wp.T
    wtb[HID + LAB] = bp
    # partition-major [128, kt, cols], split into tiles 0-1 / 2-4, fp8
    wtb = wtb.reshape(NKT, 128, MTOT).transpose(1, 0, 2)
    wtbA = np.ascontiguousarray(wtb[:, :, 0:256]).reshape(128, -1).astype(
        ml_dtypes.float8_e4m3)
    wtbB = np.ascontiguousarray(wtb[:, :, 256:MTOT]).reshape(128, -1).astype(
        ml_dtypes.float8_e4m3)

    jmat, trig, masks = _host_tables()

    in_maps = []
    for b in range(B):
        xT = np.zeros((KPAD, S), np.float32)
        xT[:HID] = hidden[b].T
        xT[HID:HID + LAB] = ent_emb[entity_labels[b]].T
        xT[HID + LAB] = 1.0
        xT = np.ascontiguousarray(
            xT.reshape(NKT, 128, S).transpose(1, 0, 2).reshape(128, -1)
        ).astype(ml_dtypes.float8_e4m3)
        in_maps.append(dict(xT=xT, wtbA=wtbA, wtbB=wtbB,
                            jmat=jmat, trig=trig, masks=masks))

    nc = _get_nc()
    res = run_bass_kernel_spmd(nc, in_maps, core_ids=list(range(NCORES)),
                               trace=_want_trace)

    # heads in device emission order: A=gt_head, B=ent h0, C=gt_tail, D=ent h1
    heads = [
        (0, 4, (4, 5), "head"),   # sums cols 0:4
        (1, 3, (0, 1), "ent0"),   # cols 4:7
        (2, 4, (6, 7), "tail"),   # cols 7:11
        (3, 3, (2, 3), "ent1"),   # cols 11:14
    ]
    gt_entity = np.asarray(gt_entity)
    gt_head = np.asarray(gt_head)
    gt_tail = np.asarray(gt_tail)
    total = 0.0
    for b in range(B):
        out = res.results[b]
        sums_v = out["sums"].astype(np.float64)    # [128, NSUM]
        qkv = out["qkout"]                         # [8, 68, 1024] bf16
        col = 0
        for _, ncols, (gq, gk), kind in heads:
            E = float(np.sum(sums_v[:, col:col + ncols]))
            col += ncols
            if kind == "head":
                gt = gt_head[b, 0]
            elif kind == "tail":
                gt = gt_tail[b, 0]
            elif kind == "ent0":
                gt = gt_entity[b, 0]
            else:
                gt = gt_entity[b, 1]
            total += _mcce_host(E, qkv[gq], qkv[gk], gt)

    if _want_trace:
        kernel._last_results = res
    return np.array(total, dtype=np.float32)


# revision 23
# speedup vs baseline: 1.2439x; 1.0325x over previous
"""Bass/Trainium2 kernel for nn_GPREDecoder (GlobalPointer relation-extraction loss).

Strategy: data-parallel over batch (B=8 -> 8 cores, 1 example per core).
Per example the device computes:
  - projT = W_all @ x_aug.T (bf16, bias folded as an extra contraction row),
    channel layout permuted so the no-rope "head" groups finish first
  - RoPE rotation (J-matmul on PE + cos/sin elementwise on DVE, bf16)
  - per-head S x S logit rounds on PE into [128, 2048] psum tiles,
    exp(SCALE*logit) on ACT with fused per-row accumulation (2048-wide spans)
  - ships per-round exp-sums and the final bf16 q/k tensors
Host gathers the 64 ground-truth pairs per head from q/k and applies the
multilabel-CE pos/neg log corrections in float64.
"""

import ml_dtypes
import numpy as np
from contextlib import ExitStack

import concourse.bass as bass
import concourse.mybir as mybir
import concourse.tile as tile
from concourse import bacc
from concourse.bass_utils import run_bass_kernel_spmd

B, S, HID, LAB = 8, 1024, 1024, 64
HD = 68
SCALE = 1.0 / HD**0.5
INF = 1.0e12
NCORES = 8
NEG_BIG = -1.0e9  # additive pre-scale mask; exp(SCALE*NEG_BIG) == 0 in fp32
KROWS = HID + LAB + 1          # 1089 real contraction rows (bias folded)
NKT = 9
KT_ROWS = [128] * 8 + [KROWS - 8 * 128]  # last kt tile has 65 live rows
KPAD = NKT * 128               # host-side padded allocation

# groups: 0=qe0 1=ke0 2=qe1 3=ke1 4=qh 5=kh 6=qt 7=kt
_GROUP_ORIG = [0, 68, 136, 204, 272, 340, 408, 476]
# projection channel layout: (wtb_col0, group, group_off, cnt)
# qh and kh are whole in tiles 0/1 so head A needs no spill assembly;
# qe0 (rope, off the early critical path) is the split group instead.
_LAYOUT = [
    (0, 4, 0, 68), (68, 0, 0, 60),
    (128, 5, 0, 68), (196, 0, 60, 8), (204, 6, 0, 52),
    (256, 1, 0, 68), (324, 6, 52, 16), (340, 7, 0, 44),
    (384, 2, 0, 68), (452, 7, 44, 24),
    (476, 3, 0, 68),
]
T_OFF = [0, 128, 256, 384, 476]   # wtb col offset per proj m-tile
T_W = [128, 128, 128, 92, 68]     # live width per proj m-tile
MTOT = 544

# spill-group assembly: dst_tile -> [(src_tile, src_row0, cnt, dst_row0)]
_ASM = {
    0: [(0, 68, 60, 0), (1, 68, 8, 60)],   # qe0
    6: [(1, 76, 52, 0), (2, 68, 16, 52)],  # qt
    7: [(2, 84, 44, 0), (3, 68, 24, 44)],  # kt
}

# full-head rounds: 4 rounds x 2048 span (2 m-tiles each)
_FULL_ROUNDS = [
    (2048,
     [(2 * x, 0, 0, 512), (2 * x, 512, 512, 512),
      (2 * x + 1, 1024, 0, 512), (2 * x + 1, 1536, 512, 512)],
     [])
    for x in range(4)
]
# tril-head rounds: (span, pieces[(m, lo, src, w)], masks[(kind, off)])
# kind 0 = tril (diag block), 1 = all -inf (m7's 128-col pad)
_TRIL_PIECES = [
    (1920, [(0, 0, 0, 1024), (1, 1024, 128, 896)],
     [(0, 0), (0, 1024)]),
    (1664, [(2, 0, 256, 768), (3, 768, 384, 640), (6, 1408, 768, 256)],
     [(0, 0), (0, 768), (0, 1408)]),
    (1152, [(4, 0, 512, 512), (5, 512, 640, 384), (7, 896, 768, 256)],
     [(0, 0), (0, 512), (1, 896), (0, 1024)]),
]


def _chunks(pieces, head=0):
    """Split round pieces at 512-col psum bank boundaries.

    head>0: emit the first `head` cols of each piece as a separate leading
    chunk (so DVE mask-adds on those blocks overlap the round's remaining
    matmuls)."""
    out = []
    rest = []
    for m, lo, src, w in pieces:
        if head:
            out.append((m, lo, src, head))
            lo, src, w = lo + head, src + head, w - head
        off = 0
        while off < w:
            n = min(512 - ((lo + off) % 512), w - off)
            rest.append((m, lo + off, src + off, n))
            off += n
    return out + rest


_TRIL_ROUNDS = [(sp, _chunks(pc, head=128), mk) for sp, pc, mk in _TRIL_PIECES]
_FULL_ROUNDS = [(sp, _chunks(pc), mk) for sp, pc, mk in _FULL_ROUNDS]

# head emission order: A=(qh,kh) full, B=(qe0,ke0) tril, C=(qt,kt) full,
# D=(qe1,ke1) tril.  sums column ranges follow emission order.
NSUM = 14


def _build_perm():
    """perm[wtb_col] = original channel index."""
    perm = np.zeros(MTOT, np.int64)
    for col0, g, goff, cnt in _LAYOUT:
        perm[col0:col0 + cnt] = np.arange(
            _GROUP_ORIG[g] + goff, _GROUP_ORIG[g] + goff + cnt)
    return perm


def _build_nc():
    f32 = mybir.dt.float32
    bf16 = mybir.dt.bfloat16
    fp8 = mybir.dt.float8e4
    Exp = mybir.ActivationFunctionType.Exp
    mult = mybir.AluOpType.mult
    add = mybir.AluOpType.add

    nc = bacc.Bacc("TRN2", target_bir_lowering=False)

    # host layouts are partition-major (one big DMA descriptor per
    # partition); per-queue DMA streaming is ~95GB/s, so inputs are fp8.
    # wtb is split: A = proj tiles 0-1 (cols 0:256), B = tiles 2-4.
    xT = nc.dram_tensor("xT", [128, NKT * S], fp8, kind="ExternalInput")
    wtbA = nc.dram_tensor("wtbA", [128, NKT * 256], fp8,
                          kind="ExternalInput")
    wtbB = nc.dram_tensor("wtbB", [128, NKT * 288], fp8,
                          kind="ExternalInput")
    jmat = nc.dram_tensor("jmat", [128, 128], bf16, kind="ExternalInput")
    trig = nc.dram_tensor("trig", [HD, 2 * S], bf16, kind="ExternalInput")
    masks = nc.dram_tensor("masks", [128, 256], f32, kind="ExternalInput")
    sums = nc.dram_tensor("sums", [128, NSUM], f32, kind="ExternalOutput")
    qkout = nc.dram_tensor("qkout", [8, HD, S], bf16, kind="ExternalOutput")

    xT_r = xT.rearrange("p (o f) -> p o f", o=NKT)      # [128, 9, 1024]
    wtbA_r = wtbA.rearrange("p (o f) -> p o f", o=NKT)  # [128, 9, 256]
    wtbB_r = wtbB.rearrange("p (o f) -> p o f", o=NKT)  # [128, 9, 288]

    with tile.TileContext(nc) as tc, ExitStack() as ctx:
        singles = ctx.enter_context(tc.tile_pool(name="singles", bufs=1))
        scratch = ctx.enter_context(tc.tile_pool(name="scratch", bufs=2))

        xT_sb = singles.tile([128, NKT, S], fp8, tag="xT_sb", name="xT_sb")
        wtbA_sb = singles.tile([128, NKT, 256], fp8, tag="wtbA_sb",
                               name="wtbA_sb")
        wtbB_sb = singles.tile([128, NKT, 288], fp8, tag="wtbB_sb",
                               name="wtbB_sb")
        jmat_sb = singles.tile([128, 128], bf16, tag="jmat_sb", name="jmat_sb")
        trig_sb = singles.tile([HD, 2 * S], bf16, tag="trig_sb",
                               name="trig_sb")
        masks_sb = singles.tile([128, 256], f32, tag="masks_sb",
                                name="masks_sb")
        dense = [singles.tile([128, S], bf16, tag=f"dense{t}",
                              name=f"dense{t}") for t in range(5)]
        asm = {g: singles.tile([HD, S], bf16, tag=f"asm{g}", name=f"asm{g}")
               for g in (0, 6, 7)}
        warm_sb = singles.tile([128, 512], bf16, tag="warm", name="warm")
        qrot = {g: singles.tile([HD, S], bf16, tag=f"qrot{g}",
                                name=f"qrot{g}") for g in (0, 1, 2, 3)}
        sums_sb = singles.tile([128, NSUM], f32, tag="sums_sb",
                               name="sums_sb")
        dummy = singles.tile([1, 8], f32, tag="dummy", name="dummy")

        tril_sb = masks_sb[:, 0:128]
        neg_sb = masks_sb[:, 128:256]
        sin_sb = trig_sb[:, 0:S]
        cos_sb = trig_sb[:, S:2 * S]
        # per proj tile: (wtb sbuf tile, col offset within it)
        wsrc = [(wtbA_sb, 0), (wtbA_sb, 128), (wtbB_sb, 0), (wtbB_sb, 128),
                (wtbB_sb, 220)]

        # matmul operand source per group (bf16 [68, S] views)
        def gsrc(g):
            if g == 1:
                return dense[2][0:HD, :]
            if g == 2:
                return dense[3][0:HD, :]
            if g == 3:
                return dense[4][0:HD, :]
            if g == 4:
                return dense[0][0:HD, :]
            if g == 5:
                return dense[1][0:HD, :]
            return asm[g][:, :]

        def gfin(g):  # post-rope operand
            return qrot[g][:, :] if g < 4 else gsrc(g)

        # Early: zero accumulators (scalar prewarm issued after its DMAs).
        nc.vector.memset(sums_sb[:], 0.0)
        nc.vector.memset(dummy[:], 0.0)
        nc.vector.memset(warm_sb[:], 0.0)

        # ---- input DMAs ------------------------------------------------
        # One descriptor per partition per chunk (contiguous in both DRAM
        # and SBUF).
        def dma_xt(eng, k0, k1):
            eng.dma_start(out=xT_sb[:, k0:k1, :], in_=xT_r[:, k0:k1, :])

        nc.sync.dma_start(out=wtbA_sb[:, 0:3], in_=wtbA_r[:, 0:3, :])
        dma_xt(nc.scalar, 2, 4)
        nc.gpsimd.dma_start(out=xT_sb[0:KT_ROWS[8], 8, :],
                            in_=xT_r[0:KT_ROWS[8], 8, :])
        dma_xt(nc.sync, 0, 2)
        dma_xt(nc.scalar, 6, 8)
        nc.sync.dma_start(out=wtbA_sb[:, 3:9], in_=wtbA_r[:, 3:9, :])
        dma_xt(nc.sync, 4, 6)
        nc.scalar.dma_start(out=trig_sb[:], in_=trig[:, :])
        nc.sync.dma_start(out=wtbB_sb[:], in_=wtbB_r[:, :, :])
        nc.gpsimd.dma_start(out=jmat_sb[:], in_=jmat[:, :])
        nc.gpsimd.dma_start(out=masks_sb[:], in_=masks[:, :])
        # pre-warm the ACT exp table while input DMAs stream
        nc.scalar.activation(dummy[:], dummy[:], Exp)

        pool_h = ctx.enter_context(
            tc.tile_pool(name="ph", bufs=1, space="PSUM"))

        acc_col = [0]

        def head_round(pools, q, k, span, chunks, mks, label):
            ph = pools[0].tile([128, 2048], f32, tag=f"ph{pools[1]}",
                               name=label)
            for (m, lo, src, n) in chunks:
                nc.tensor.matmul(
                    ph[:, lo:lo + n],
                    q[:, m * 128:(m + 1) * 128],
                    k[:, src:src + n],
                    start=True, stop=True,
                )
            for kind, off in mks:
                msk = tril_sb if kind == 0 else neg_sb
                nc.vector.tensor_tensor(ph[:, off:off + 128],
                                        ph[:, off:off + 128], msk, add)
            nc.scalar.activation(
                ph[:, 0:span], ph[:, 0:span], Exp, scale=SCALE,
                accum_out=sums_sb[:, acc_col[0]:acc_col[0] + 1])
            acc_col[0] += 1

        with tc.tile_pool(name="pp", bufs=2, space="PSUM") as pool_p:

            def proj_tile(t):
                pt = pool_p.tile([128, S], f32, tag="pp", name=f"proj{t}")
                w = T_W[t]
                wsb, woff = wsrc[t]
                for kt in range(NKT):
                    r = KT_ROWS[kt]
                    for c in (0, 512):
                        nc.tensor.matmul(
                            pt[0:w, c:c + 512],
                            wsb[0:r, kt, woff:woff + w],
                            xT_sb[0:r, kt, c:c + 512],
                            start=(kt == 0), stop=(kt == NKT - 1),
                        )
                return pt

            def proj_pair(t0, t1):
                pa = pool_p.tile([128, S], f32, tag="pp", name=f"proj{t0}")
                pb = pool_p.tile([128, S], f32, tag="pp", name=f"proj{t1}")
                for kt in range(NKT):
                    r = KT_ROWS[kt]
                    for t, pt in ((t0, pa), (t1, pb)):
                        w = T_W[t]
                        wsb, woff = wsrc[t]
                        for c in (0, 512):
                            nc.tensor.matmul(
                                pt[0:w, c:c + 512],
                                wsb[0:r, kt, woff:woff + w],
                                xT_sb[0:r, kt, c:c + 512],
                                start=(kt == 0), stop=(kt == NKT - 1),
                            )
                return pa, pb

            def evac(t, pt, eng="dve"):
                w = T_W[t]
                if eng == "act":
                    nc.scalar.copy(out=dense[t][0:w, :], in_=pt[0:w, :])
                else:
                    nc.vector.tensor_copy(out=dense[t][0:w, :],
                                          in_=pt[0:w, :])

            def assemble(g, eng):
                for (st, r0, cnt, d0) in _ASM[g]:
                    eng.dma_start(out=asm[g][d0:d0 + cnt, :],
                                  in_=dense[st][r0:r0 + cnt, :])

            def jrot(src, r):
                pj = pool_p.tile([128, S], f32, tag="pp", name="j")
                for c in (0, 512):
                    nc.tensor.matmul(pj[:, c:c + 512], jmat_sb[0:r, :],
                                     src[0:r, c:c + 512],
                                     start=True, stop=True)
                return pj

            def rope(g, pj):
                rtmp = scratch.tile([HD, S], bf16, tag="rtmp",
                                    name=f"rtmp{g}")
                nc.vector.tensor_tensor(rtmp[:, :], pj[0:HD, :], sin_sb,
                                        mult)
                nc.vector.tensor_tensor(qrot[g][:, :], gsrc(g), cos_sb, mult)
                nc.vector.tensor_tensor(qrot[g][:, :], qrot[g][:, :],
                                        rtmp[:, :], add)

            # ---- PE warm-up: dummy matmuls during the input DMA wait ----
            # gets the HAM clock gate to 8/8 (~3.4us busy) so the real
            # projection streams at 2.4GHz from its first matmul
            pw = pool_p.tile([128, S], f32, tag="pp", name="warmup")
            for _ in range(8):
                nc.tensor.matmul(pw[:, 0:512], warm_sb[:, 0:128],
                                 warm_sb[:, 0:512], start=True, stop=True)

            # ---- phase 1: proj tiles 0,1 (qh | kh + qe0/qt spill) -------
            pa, pb = proj_pair(0, 1)
            evac(0, pa, "act")
            evac(1, pb)
            assemble(0, nc.sync)                       # qe0
            nc.gpsimd.dma_start(out=qkout[4], in_=dense[0][0:HD, :])  # qh
            nc.gpsimd.dma_start(out=qkout[5], in_=dense[1][0:HD, :])  # kh

            # ---- head A (qh x kh, full) round 0 -------------------------
            qA, kA = gfin(4), gfin(5)
            sp, ch, mk = _FULL_ROUNDS[0]
            head_round((pool_h, 0), qA, kA, sp, ch, mk, "A0")

            # ---- proj tile 2 (ke0 + qt/kt spill) ------------------------
            pt2 = proj_tile(2)
            evac(2, pt2)
            assemble(6, nc.sync)                       # qt
            nc.gpsimd.dma_start(out=qkout[6], in_=asm[6][:, :])       # qt

            sp, ch, mk = _FULL_ROUNDS[1]
            head_round((pool_h, 0), qA, kA, sp, ch, mk, "A1")

            pj0 = jrot(asm[0], HD)                     # J(qe0)
            rope(0, pj0)
            nc.gpsimd.dma_start(out=qkout[0], in_=qrot[0][:, :])      # qe0r

            sp, ch, mk = _FULL_ROUNDS[2]
            head_round((pool_h, 0), qA, kA, sp, ch, mk, "A2")

            pj1 = jrot(dense[2], HD)                   # J(ke0)
            rope(1, pj1)
            nc.gpsimd.dma_start(out=qkout[1], in_=qrot[1][:, :])      # ke0r

            # ---- proj tile 3 (qe1 + kt spill) ---------------------------
            pt3 = proj_tile(3)
            evac(3, pt3)
            assemble(7, nc.sync)                       # kt
            nc.gpsimd.dma_start(out=qkout[7], in_=asm[7][:, :])       # kt

            sp, ch, mk = _FULL_ROUNDS[3]
            head_round((pool_h, 0), qA, kA, sp, ch, mk, "A3")

            # ---- head B (qe0 x ke0, tril) round 0 + proj tile 4 ---------
            qB, kB = gfin(0), gfin(1)
            sp, ch, mk = _TRIL_ROUNDS[0]
            head_round((pool_h, 0), qB, kB, sp, ch, mk, "B0")

            pt4 = proj_tile(4)
            evac(4, pt4)

            sp, ch, mk = _TRIL_ROUNDS[1]
            head_round((pool_h, 0), qB, kB, sp, ch, mk, "B1")

            pj2 = jrot(dense[3], HD)                   # J(qe1)
            rope(2, pj2)
            nc.sync.dma_start(out=qkout[2], in_=qrot[2][:, :])        # qe1r

            sp, ch, mk = _TRIL_ROUNDS[2]
            head_round((pool_h, 0), qB, kB, sp, ch, mk, "B2")

            pj3 = jrot(dense[4], HD)                   # J(ke1)
            rope(3, pj3)
            nc.sync.dma_start(out=qkout[3], in_=qrot[3][:, :])        # ke1r

            nc.sync.dma_start(out=sums[:, 0:7], in_=sums_sb[:, 0:7])

        # proj pool released; open a second 2048 pool for double buffering
        with tc.tile_pool(name="ph2", bufs=1, space="PSUM") as pool_h2:
            qC, kC = gfin(6), gfin(7)
            qD, kD = gfin(2), gfin(3)
            tail = ([(qC, kC, r) for r in _FULL_ROUNDS]
                    + [(qD, kD, r) for r in _TRIL_ROUNDS])
            for i, (q, k, (sp, ch, mk)) in enumerate(tail):
                pool = (pool_h, 0) if i % 2 == 0 else (pool_h2, 1)
                head_round(pool, q, k, sp, ch, mk, f"T{i}")

        nc.sync.dma_start(out=sums[:, 7:NSUM], in_=sums_sb[:, 7:NSUM])
        assert acc_col[0] == NSUM

    nc.finalize()
    return nc


_NC_CACHE = None


def _get_nc():
    global _NC_CACHE
    if _NC_CACHE is None:
        _NC_CACHE = _build_nc()
    return _NC_CACHE


def _host_tables():
    pos = np.arange(S, dtype=np.float64)[:, None]
    inv = np.power(10000.0, -2.0 * np.arange(HD // 2, dtype=np.float64) / HD)
    ang = pos * inv                                   # [S, 34]
    trig = np.zeros((HD, 2 * S), np.float32)
    trig[:, 0:S] = np.repeat(np.sin(ang), 2, axis=1).T
    trig[:, S:2 * S] = np.repeat(np.cos(ang), 2, axis=1).T
    trig = trig.astype(ml_dtypes.bfloat16)
    jmat = np.zeros((128, 128), np.float32)
    for i in range(HD // 2):
        # J[2i, 2i+1] = -1 ; J[2i+1, 2i] = +1  -> stored transposed
        jmat[2 * i + 1, 2 * i] = -1.0
        jmat[2 * i, 2 * i + 1] = 1.0
    jmat = jmat.astype(ml_dtypes.bfloat16)
    masks = np.zeros((128, 256), np.float32)
    masks[:, 0:128] = np.where(
        np.arange(128)[None, :] >= np.arange(128)[:, None], 0.0, NEG_BIG)
    masks[:, 128:256] = NEG_BIG
    return jmat, trig, masks


def _mcce_host(E_dev, q, k, gt):
    """pos/neg multilabel-CE for one (example, head). q,k: [68,S]; gt: [P,2]."""
    i = gt[:, 0].astype(np.int64)
    j = gt[:, 1].astype(np.int64)
    flat = i * S + j
    lv = np.sum(q[:, i].astype(np.float64) * k[:, j].astype(np.float64),
                axis=0) * SCALE                       # [P]
    live = flat != 0
    pos_loss = np.log1p(np.sum(np.exp(-lv[live])))
    l00 = float(np.sum(q[:, 0].astype(np.float64) * k[:, 0].astype(np.float64))
                * SCALE)
    uf, ui = np.unique(flat, return_index=True)
    keep = uf != 0
    excl = np.exp(l00) + np.sum(np.exp(lv[ui[keep]]))
    neg_loss = np.log1p(E_dev - excl)
    return pos_loss + neg_loss


def _reference_numpy(hidden, entity_labels, attention_mask, gt_entity, gt_head,
                     gt_tail, ent_emb, W_ent, b_ent, W_head, b_head, W_tail,
                     b_tail):
    """Slow exact numpy fallback (used only if attention_mask is not all-ones)."""
    x = np.concatenate([hidden, ent_emb[entity_labels]], axis=-1)

    def rope(v):
        b, s, h, d = v.shape
        pos = np.arange(s, dtype=np.float32)[:, None]
        inv = np.power(10000.0, -2.0 * np.arange(d // 2, dtype=np.float32) / d)
        ang = pos * inv
        sin = np.repeat(np.sin(ang), 2, axis=-1)[None, :, None, :]
        cos = np.repeat(np.cos(ang), 2, axis=-1)[None, :, None, :]
        v2 = np.stack([-v[..., 1::2], v[..., ::2]], axis=-1).reshape(v.shape)
        return v * cos + v2 * sin

    def gp(x, W, b, mask, heads, use_rope, tril):
        bx, sx, _ = x.shape
        proj = (x @ W.T + b).reshape(bx, sx, heads, 2 * HD)
        qw, kw = proj[..., :HD], proj[..., HD:]
        if use_rope:
            qw, kw = rope(qw), rope(kw)
        logits = np.einsum('bmhd,bnhd->bhmn', qw, kw) * SCALE
        pad = mask[:, None, None, :]
        logits = logits * pad - (1.0 - pad) * INF
        if tril:
            logits = logits - np.tril(np.ones((sx, sx), np.float32), -1) * INF
        return logits

    def mcce(y_true, y_pred):
        bx, hx, sx, _ = y_pred.shape
        flat = y_true[..., 0].astype(np.int64) * sx + y_true[..., 1]
        yp = y_pred.reshape(bx, hx, sx * sx).astype(np.float64)
        total = 0.0
        for b in range(bx):
            for h in range(hx):
                f = flat[b, h]
                live = f != 0
                lv = yp[b, h][f]
                pos = np.log1p(np.sum(np.exp(-lv[live])))
                neg_terms = yp[b, h].copy()
                neg_terms[0] = -np.inf
                neg_terms[np.unique(f)] = -np.inf
                neg = np.log1p(np.sum(np.exp(neg_terms)))
                total += pos + neg
        return total

    loss = 0.0
    loss += mcce(gt_entity, gp(x, W_ent, b_ent, attention_mask, 2, True, True))
    loss += mcce(gt_head, gp(x, W_head, b_head, attention_mask, 1, False, False))
    loss += mcce(gt_tail, gp(x, W_tail, b_tail, attention_mask, 1, False, False))
    return np.array(loss, dtype=np.float32)


def kernel(hidden, entity_labels, attention_mask, gt_entity, gt_head, gt_tail,
           ent_emb, W_ent, b_ent, W_head, b_head, W_tail, b_tail,
           _want_trace=False):
    hidden = np.asarray(hidden, np.float32)
    entity_labels = np.asarray(entity_labels)
    attention_mask = np.asarray(attention_mask, np.float32)
    ent_emb = np.asarray(ent_emb, np.float32)

    if not np.all(attention_mask == 1.0):
        return _reference_numpy(
            hidden, entity_labels, attention_mask, np.asarray(gt_entity),
            np.asarray(gt_head), np.asarray(gt_tail), ent_emb,
            np.asarray(W_ent, np.float32), np.asarray(b_ent, np.float32),
            np.asarray(W_head, np.float32), np.asarray(b_head, np.float32),
            np.asarray(W_tail, np.float32), np.asarray(b_tail, np.float32))

    W_all = np.concatenate(
        [np.asarray(W_ent, np.float32), np.asarray(W_head, np.float32),
         np.asarray(W_tail, np.float32)], axis=0)       # [544, 1088]
    b_all = np.concatenate(
        [np.asarray(b_ent, np.float32), np.asarray(b_head, np.float32),
         np.asarray(b_tail, np.float32)], axis=0)       # [544]
    perm = _build_perm()
    Wp, bp = W_all[perm], b_all[perm]
    wtb = np.zeros((KPAD, MTOT), np.float32)
    wtb[:HID + LAB] = Wp.T
    wtb[HID + LAB] = bp
    # partition-major [128, kt, cols], split into tiles 0-1 / 2-4, fp8
    wtb = wtb.reshape(NKT, 128, MTOT).transpose(1, 0, 2)
    wtbA = np.ascontiguousarray(wtb[:, :, 0:256]).reshape(128, -1).astype(
        ml_dtypes.float8_e4m3)
    wtbB = np.ascontiguousarray(wtb[:, :, 256:MTOT]).reshape(128, -1).astype(
        ml_dtypes.float8_e4m3)

    jmat, trig, masks = _host_tables()

    in_maps = []
    for b in range(B):
        xT = np.zeros((KPAD, S), np.float32)
        xT[:HID] = hidden[b].T
        xT[HID:HID + LAB] = ent_emb[entity_labels[b]].T
        xT[HID + LAB] = 1.0
        xT = np.ascontiguousarray(
            xT.reshape(NKT, 128, S).transpose(1, 0, 2).reshape(128, -1)
        ).astype(ml_dtypes.float8_e4m3)
        in_maps.append(dict(xT=xT, wtbA=wtbA, wtbB=wtbB,
                            jmat=jmat, trig=trig, masks=masks))

    nc = _get_nc()
    res = run_bass_kernel_spmd(nc, in_maps, core_ids=list(range(NCORES)),
                               trace=_want_trace)

    # heads in device emission order: A=gt_head, B=ent h0, C=gt_tail, D=ent h1
    heads = [
        (0, 4, (4, 5), "head"),   # sums cols 0:4
        (1, 3, (0, 1), "ent0"),   # cols 4:7
        (2, 4, (6, 7), "tail"),   # cols 7:11
        (3, 3, (2, 3), "ent1"),   # cols 11:14
    ]
    gt_entity = np.asarray(gt_entity)
    gt_head = np.asarray(gt_head)
    gt_tail = np.asarray(gt_tail)
    total = 0.0
    for b in range(B):
        out = res.results[b]
        sums_v = out["sums"].astype(np.float64)    # [128, NSUM]
        qkv = out["qkout"]                         # [8, 68, 1024] bf16
        col = 0
        for _, ncols, (gq, gk), kind in heads:
            E = float(np.sum(sums_v[:, col:col + ncols]))
            col += ncols
            if kind == "head":
                gt = gt_head[b, 0]
            elif kind == "tail":
                gt = gt_tail[b, 0]
            elif kind == "ent0":
                gt = gt_entity[b, 0]
            else:
                gt = gt_entity[b, 1]
            total += _mcce_host(E, qkv[gq], qkv[gk], gt)

    if _want_trace:
        kernel._last_results = res
    return np.array(total, dtype=np.float32)
